# revision 74
# baseline (speedup 1.0000x reference)
"""Softclamped multi-head attention (B=2, N=2048, DIM=1024, 16 heads x 64) on
8 TRN2 NeuronCores.

Sharding: tensor-parallel over heads - 2 heads per core. Each core computes its
heads' Q/K/V projections, attention, and a partial output projection; the 8
fp32 partials are summed on the host (the out-proj contraction dim is sharded),
so the device graph needs no collectives.

v2 structure (~287us -> target <200us):
  - rmsnorm is folded on the HOST: tokens are pre-scaled by rsqrt(mean(x^2))
    before the bf16 cast, killing the device-side sumsq matmuls, rsqrt and
    the v-epilogue multiply.
  - The softmax nonlinearity exp(6.25*tanh(sim/50)) is split across two
    engine paths per granule-PAIR:
      A-path (DVE): custom op TANH2Y computes Y = 2^23*log2(e)*6.25*
        tanh(s/50) as a deg-7 odd poly (input pre-scaled by lam so the
        sigma^7 coeff is exactly -1 => 3 scalar consts); custom op EXP2V
        turns Y into the f32 BIT PATTERN of 2^(y+0.5) via a magic-add
        round (n23 = (Y+M)-M at 2^23 granularity) + quadratic mantissa
        fit, written through the DVE's f32->u32 value conversion. The PV
        matmul reads the u32 tile's HIGH HALF-WORDS as a stride-2 bf16
        AP - bf16 bits are the f32 top 16 bits, so no engine ever runs
        a real exp.
      B-path (ACT): plain Tanh then Exp(6.25*t + 0.5*ln2) table ops,
        matching the A-path's sqrt(2) scale exactly.
    Both paths write per-PAIR [128, 2048] tiles; the pattern alternates
    pairs so DVE and ACT stay balanced.
  - the token DMA streams batch-major so all 8 batch-0 k/q projection
    groups pace per-sub-chunk in 8 PSUM banks during the load; batch-1
    projections + norm chains and all v-projection groups drain as
    scheduled prework items INSIDE phase D (batch-1 rsqrts batched
    back-to-back to bound ACT-table reloads).
  - v transposes are PE transposes into spare columns of the v-group's
    own PSUM slot + DVE copies (dma_start_transpose measured 1.2us per
    128x128 block of hwdge-queue time - too slow).
  - normalize muls ride the Pool (gpsimd) engine, which otherwise idles;
    po exits stay on ACT (moving any to DVE measured worse).

HW facts this relies on (probed on silicon in this session):
  - custom DVE f32->u32 output conversion is exact value truncation;
    magic-add rounding is IEEE RNE; full-tile Src1 works (only [P,1]
    broadcast Src1 is broken).
  - a matmul moving operand can be a stride-2 bf16 view of a u32 tile.
  - matmul dtype mixing 16/32-bit is rejected by walrus (hence the
    half-word trick instead of f32r).
  - shift/divide AluOps return 0 on the TRN2 DVE (hence the magic-add
    construction instead of exponent shifts).
  - gpsimd/Pool has no PSUM port; PSUM exits must use ACT/DVE.
"""

import os
os.environ.setdefault("JAX_PLATFORMS", "axon")
import sys
if "/opt/trn_rl_repo" not in sys.path:
    sys.path.insert(0, "/opt/trn_rl_repo")

import numpy as np
import ml_dtypes

import concourse.bass as bass  # noqa: F401
from concourse import bacc, mybir
import concourse.tile as tile
from concourse.bass_utils import run_bass_kernel_spmd

B, N, DIM = 2, 2048, 1024
H, DH = 16, 64
NCORES = 8
HPC = H // NCORES          # heads per core = 2
CD = HPC * DH              # per-core projection width = 128
T = B * N                  # 4096 tokens
DCH = DIM // 128           # 8 dim chunks
F32 = mybir.dt.float32
F16 = mybir.dt.float16
BF16 = mybir.dt.bfloat16
U32 = mybir.dt.uint32
AF = mybir.ActivationFunctionType
IB = 512                   # attention i-block (queries per phase-D block)
NBLK = T // IB             # 8 blocks
NJ = N // 128              # 16 key chunks per batch
NG = NBLK * NJ             # 128 granules, 64 pairs

SOFTCLAMP = 50.0
RMS_EPS = 1e-6

# ---------------------------------------------------------------------------
# Custom-op numerics (see fit_ops.py for derivation + numpy validation).
# ---------------------------------------------------------------------------
AMP = 2.0**23 * np.log2(np.e) * 6.25
TA, TB_, TC, TD = 0.99819183, -0.31795733, 0.09607557, -0.01496778
_a1 = AMP * TA / 50.0
_a3 = AMP * TB_ / 50.0**3
_a5 = AMP * TC / 50.0**5
_a7 = AMP * TD / 50.0**7
LAM = float((-_a7) ** (1.0 / 7.0))   # sigma = LAM*s ; sigma^7 coeff == -1
K5 = float(np.float32(_a5 / LAM**5))
K3 = float(np.float32(_a3 / LAM**3))
K1 = float(np.float32(_a1 / LAM))

Q0, Q1, Q2 = 1.41569374, 0.9921173, 0.3220771   # 2^(f+0.5) quad minimax
M_BIG = float(np.float32(1.5 * 2.0**46))
# +2^15 nudges the bf16 high-halfword truncation to round-to-nearest
B0 = float(np.float32(2.0**23 * (126.0 + Q0) + 32768.0))
B1 = float(np.float32(Q1))
B2 = float(np.float32(Q2 / 2.0**23))
EXP_B_BIAS = float(0.5 * np.log(2.0))   # B-path: w = exp(6.25*t + 0.5*ln2)

_OPS = None


def _register_ops():
    global _OPS
    if _OPS is not None:
        return _OPS
    import concourse.dve_ops as dve_ops
    from concourse.dve_spec import (Spec, Src0, Src1, C0, C1, C2, lower,
                                    _has_src1)
    from concourse.dve_uop import DveOpSpec

    def mk(name, body, ref):
        if name in dve_ops._SUB_OPCODE_FOR_NAME:
            return next(o for o in dve_ops.OPS if o.name == name)
        spec = Spec(body=body, reference=ref)
        row = dve_ops._CUSTOM_DVE_ROW_BASE + len(dve_ops.OPS)
        assert row < 0x20
        dve_ops._SUB_OPCODE_FOR_NAME[name] = row
        shas = {}
        for ver in ("v3", "v4"):
            uops = lower(spec, ver=ver)
            shas[ver] = DveOpSpec(name=name, opcode=row, uops=uops,
                                  rd1_en=_has_src1(spec)).sha(ver)
        op = dve_ops.DveOp(name, spec, subdim=False, uops_sha=shas)
        dve_ops.OPS.append(op)
        dve_ops.CUSTOM_DVE_SPECS[name] = op.spec
        return op

    u = Src0 * Src0
    body1 = ((((C0 - u) * u + C1) * u) + C2) * Src0

    def ref1(in0, in1, s0, s1, imm2):
        x = in0.astype(np.float32)
        uu = x * x
        return ((((s0 - uu) * uu + s1) * uu) + imm2) * x

    n23 = (Src0 + C0) - C0
    Fv = Src0 - n23
    body2 = (((Fv * C1) + C2) * Fv + Src1) + n23

    def ref2(in0, in1, s0, s1, imm2):
        Y = in0.astype(np.float32)
        nn = ((Y + s0).astype(np.float32) - s0).astype(np.float32)
        Ff = (Y - nn).astype(np.float32)
        t = ((Ff * s1).astype(np.float32) + imm2).astype(np.float32)
        t = ((t * Ff).astype(np.float32) + in1).astype(np.float32)
        return (t + nn).astype(np.float32)

    _OPS = (mk("TANH2Y_ANT", body1, ref1), mk("EXP2V_ANT", body2, ref2))
    return _OPS


def build_nc(debug_outs=False):
    op1, op2 = _register_ops()
    nc = bacc.Bacc("TRN2", target_bir_lowering=False, debug=False,
                   num_devices=NCORES)
    tok = nc.declare_dram_parameter("tok", [DIM, T], BF16, isOutput=False)
    wq = nc.declare_dram_parameter("wq", [128, DCH * CD], BF16, isOutput=False)
    wk = nc.declare_dram_parameter("wk", [128, DCH * CD], BF16, isOutput=False)
    wv = nc.declare_dram_parameter("wv", [128, DCH * CD], BF16, isOutput=False)
    wo = nc.declare_dram_parameter("wo", [CD, DIM], BF16, isOutput=False)
    # per-partition scale for the q-norm rsqrt: 1/(g2*LAM)^2
    gq = nc.declare_dram_parameter("gq", [CD, 1], F32, isOutput=False)
    out = nc.declare_dram_parameter("out", [T, DIM], BF16, isOutput=True)
    dbg = None
    if debug_outs:
        dbg = {
            "d_qT": nc.declare_dram_parameter("d_qT", [128, T], BF16, True),
            "d_kT": nc.declare_dram_parameter("d_kT", [128, T], BF16, True),
            "d_v": nc.declare_dram_parameter("d_v", [128, T // 128, 256],
                                             BF16, True),
            "d_exA": nc.declare_dram_parameter("d_exA", [128, 2048], U32, True),
            "d_exB": nc.declare_dram_parameter("d_exB", [128, 2048], BF16, True),
        }

    with tile.TileContext(nc) as tc:
        _emit(nc, tc, op1, op2, tok, wq, wk, wv, wo, gq, out, dbg)
    nc.compile()
    return nc


def _emit(nc, tc, op1, op2, tok, wq, wk, wv, wo, gq, out, dbg=None):
    with tc.tile_pool(name="const", bufs=1) as const, \
         tc.tile_pool(name="core", bufs=1) as core:

        # ---- constants / weights ----
        from concourse.masks import make_identity
        ones_bf = const.tile([128, 128], BF16, tag="ones")
        nc.vector.memset(ones_bf[:], 1.0)
        ident = const.tile([128, 128], BF16, tag="ident")
        make_identity(nc, ident[:])
        bias0 = const.tile([128, 1], F32, tag="bias0")
        nc.vector.memset(bias0[:], 0.0)
        bias_e = const.tile([128, 1], F32, tag="bias_e")
        nc.vector.memset(bias_e[:], EXP_B_BIAS)
        b0t = const.tile([128, 2 * 2 * IB], F32, tag="b0t")
        nc.vector.memset(b0t[:], B0)
        gq_sb = const.tile([128, 1], F32, tag="gq")
        nc.scalar.dma_start(out=gq_sb[:], in_=gq[:])
        wq_sb = const.tile([128, DCH, CD], BF16, tag="wq")
        wk_sb = const.tile([128, DCH, CD], BF16, tag="wk")
        wv_sb = const.tile([128, DCH, CD], BF16, tag="wv")
        # wk/wq ride the sync queue AHEAD of the token stream so the first
        # projection matmuls aren't starved behind 8 MB of tokens
        for w_dram, w_sb in ((wk, wk_sb), (wq, wq_sb)):
            nc.sync.dma_start(out=w_sb[:],
                              in_=w_dram.rearrange("p (c m) -> p c m",
                                                   c=DCH))
        nc.scalar.dma_start(out=wv_sb[:],
                            in_=wv.rearrange("p (c m) -> p c m", c=DCH))
        wo_sb = const.tile([128, DIM], BF16, tag="wo")
        nc.scalar.dma_start(out=wo_sb[:], in_=wo[:])

        # persistent tensors
        qT = core.tile([128, T], BF16, tag="qT")
        kT = core.tile([128, T], BF16, tag="kT")
        vT = core.tile([128, T], BF16, tag="vT")
        # augmented v, per 128-token chunk: [onesA(64)|vA(64)|vB(64)|onesB(64)]
        v_sb = core.tile([128, T // 128, 256], BF16, tag="v")
        nc.vector.memset(v_sb[:, :, 0:64], 1.0)
        nc.vector.memset(v_sb[:, :, 192:256], 1.0)

        tok_ch = [core.tile([128, T], BF16, tag=f"tok{ch}", name=f"tok{ch}")
                  for ch in range(DCH)]
        # batch-major sub-chunk stream: each (b, ch) arrival unleashes the
        # 8 resident projection groups' ch-matmuls for that batch, so the
        # whole q+k projection pass is DMA-paced in 8 PSUM banks.
        for b in range(B):
            nsl = slice(b * N, (b + 1) * N)
            for ch in range(DCH):
                # alternate hwdge queues: each runs at ~83% descriptor
                # efficiency, so two in parallel approach the full HBM rate
                q = nc.sync if ch % 2 == 0 else nc.scalar
                q.dma_start(out=tok_ch[ch][:, nsl],
                            in_=tok[ch * 128:(ch + 1) * 128, nsl])

        # ------------------------------------------------------------------
        # projection-group + head-norm emitters (psum pool passed per phase)
        # ------------------------------------------------------------------
        def proj_group(ps_pool, w_sb, dstT, th, name, copy_dve=False):
            tsl = slice(th * 512, (th + 1) * 512)
            pq = ps_pool.tile([128, 2 * IB], F32, tag="sim", name=name)
            for ch in range(DCH):
                nc.tensor.matmul(pq[:, 0:512], w_sb[:, ch, :],
                                 tok_ch[ch][:, tsl], start=(ch == 0),
                                 stop=True, skip_group_check=True)
            if copy_dve:
                nc.vector.tensor_copy(dstT[:, tsl], pq[:, 0:512])
            else:
                nc.scalar.activation(dstT[:, tsl], pq[:, 0:512], AF.Copy)
            return pq

        def norm_chains_batched(ps_pool, pc_pool, specs, psum_shape=2 * IB,
                                tag="sim"):
            # specs: list of (dstT, tp, is_q, name). All squ muls first,
            # then all n2 matmuls, then rsqrts, then the in-place muls —
            # each engine's FIFO pipelines instead of ping-ponging.
            # n2[p, t] = sum of squares over the head owning partition p.
            sq_tiles, rq_tiles, n2s = [], [], []
            for dstT, tp, is_q, name in specs:
                tfull = slice(tp * 1024, (tp + 1) * 1024)
                squ = pc_pool.tile([128, 1024], BF16, tag="squ")
                nc.vector.tensor_mul(squ[:], dstT[:, tfull], dstT[:, tfull])
                sq_tiles.append(squ)
            for squ, (dstT, tp, is_q, name) in zip(sq_tiles, specs):
                pair = []
                for ti in range(2):
                    csl = slice(ti * 512, (ti + 1) * 512)
                    n2 = ps_pool.tile([128, psum_shape], F32, tag=tag,
                                      name=f"{name}_{ti}")
                    nc.tensor.matmul(n2[0:64, 0:512], ones_bf[0:64, 0:64],
                                     squ[0:64, csl], start=True, stop=True)
                    nc.tensor.matmul(n2[64:128, 0:512], ones_bf[64:128, 0:64],
                                     squ[64:128, csl], start=True, stop=True)
                    pair.append(n2)
                n2s.append(pair)
            for pair, (dstT, tp, is_q, name) in zip(n2s, specs):
                rq = pc_pool.tile([128, 1024], F16, tag="rq")
                sc = gq_sb[:] if is_q else 1.0
                for ti in range(2):
                    csl = slice(ti * 512, (ti + 1) * 512)
                    nc.scalar.activation(rq[:, csl], pair[ti][:, 0:512],
                                         AF.Abs_reciprocal_sqrt,
                                         bias=bias0[:], scale=sc)
                rq_tiles.append(rq)
            for rq, (dstT, tp, is_q, name) in zip(rq_tiles, specs):
                tfull = slice(tp * 1024, (tp + 1) * 1024)
                nc.vector.tensor_mul(dstT[:, tfull], dstT[:, tfull], rq[:])

        def vproj_mm(ps_pool, th, name):
            return proj_group(ps_pool, wv_sb, vT, th, name)

        def vtr(pq, i, tv):
            # transposes ride unused columns of the group's own pq slot
            ptr = pq[:, 512 + i * 64: 512 + (i + 1) * 64].bitcast(BF16)
            nc.tensor.transpose(ptr,
                                vT[:, tv * 128:(tv + 1) * 128], ident[:])
            nc.vector.tensor_copy(v_sb[:, tv, 64:192].bitcast(U32),
                                  ptr.bitcast(U32))

        # ---- lead-in: ALL q/k projections + head norms, chunk-major with
        # 8 resident accumulation banks per batch pass, so PE work is paced
        # by the token DMA stream. Keeps the Abs_reciprocal_sqrt ACT-table
        # state out of phase D (tanh/exp/copy share one table set). ----
        # ---- lead-in: BATCH-0 q/k projections + head norms only. Batch-1
        # projections and norms drain inside phase D (its first 4 blocks
        # touch only batch 0), with their rsqrts batched back-to-back so
        # the tanh/exp ACT-table is reloaded exactly twice. ----
        with tc.tile_pool(name="psc", bufs=8, space="PSUM") as psc, \
             tc.tile_pool(name="pc0", bufs=2) as pc0:
            # PE HAM warm-up: dummy matmuls before the first token
            # sub-chunk lands, so the projection stream runs at 2.4 GHz
            warm = psc.tile([128, 512], F32, tag="pp", name="warm")
            for _ in range(30):
                nc.tensor.matmul(warm[:, 0:128], ones_bf[:], ident[:],
                                 start=True, stop=True,
                                 skip_group_check=True)
            pqk = {}
            for th in range(4):
                pqk["k", th] = psc.tile([128, 512], F32, tag="pp",
                                        name=f"pk{th}")
                pqk["q", th] = psc.tile([128, 512], F32, tag="pp",
                                        name=f"pq{th}")
            for ch in range(DCH):
                for th in range(4):
                    tsl = slice(th * 512, (th + 1) * 512)
                    nc.tensor.matmul(pqk["k", th][:], wk_sb[:, ch, :],
                                     tok_ch[ch][:, tsl], start=(ch == 0),
                                     stop=True, skip_group_check=True)
                    nc.tensor.matmul(pqk["q", th][:], wq_sb[:, ch, :],
                                     tok_ch[ch][:, tsl], start=(ch == 0),
                                     stop=True, skip_group_check=True)
            # k copies on ACT, q copies on the otherwise-idle DVE so the
            # two copy streams drain in parallel ahead of the chains
            for th in range(4):
                tsl = slice(th * 512, (th + 1) * 512)
                nc.scalar.activation(kT[:, tsl], pqk["k", th][:], AF.Copy)
                nc.vector.tensor_copy(qT[:, tsl], pqk["q", th][:])
            norm_chains_batched(
                psc, pc0,
                [(kT, 0, False, "kn0"), (qT, 0, True, "qn0"),
                 (kT, 1, False, "kn1"), (qT, 1, True, "qn1")],
                psum_shape=512, tag="pp")
            # prefetch the tanh+exp table set before phase D (Tanh then Exp
            # back-to-back forces a set containing both: exp_and_others)
            scr = pc0.tile([128, 16], F32, tag="scr")
            nc.scalar.activation(scr[:], ones_bf[:, 0:16], AF.Tanh,
                                 bias=bias0[:], scale=1.0)
            nc.scalar.activation(scr[:], ones_bf[:, 0:16], AF.Exp,
                                 bias=bias0[:], scale=1.0)

        # ---- phase D: attention, software-pipelined ----
        with tc.tile_pool(name="psO", bufs=1, space="PSUM") as psO, \
             tc.tile_pool(name="psS", bufs=3, space="PSUM") as psS, \
             tc.tile_pool(name="pc", bufs=2) as pc, \
             tc.tile_pool(name="pY", bufs=2) as pY, \
             tc.tile_pool(name="pxa", bufs=2) as pxa, \
             tc.tile_pool(name="pxb", bufs=2) as pxb, \
             tc.tile_pool(name="pe", bufs=1) as pe:

            # prework schedule: granule index -> emission item. v-groups for
            # batch 0 first (PV deadline g=5..20), then batch-1 q/k
            # projections + norms (sim deadline g=58), then batch-1 v
            # (PV deadline g=69+). Transpose/copy pairs trail their group's
            # matmuls so the DVE copy never head-blocks on a fresh
            # transpose.
            prework = {}

            def sched_vgroup(th, g0):
                # mm-group at g0, all 4 transpose+copy pairs at g0+1 (the pq
                # slot is recycled by the sim allocated 2 granules later, so
                # every read must be emitted by then)
                state = {}

                def mm():
                    state["pq"] = vproj_mm(psS, th, f"v{th}")

                def trs():
                    for i in range(4):
                        vtr(state["pq"], i, th * 4 + i)
                prework[g0] = mm
                prework[g0 + 1] = trs
                return g0 + 2

            g0 = 0
            for th in range(4):
                g0 = sched_vgroup(th, g0)
            # batch-1 k/q projection groups, 1 item per 2 granules
            for i, (w_sb, dstT, th, nm) in enumerate(
                    [(wk_sb, kT, th, f"k{th}") for th in range(4, 8)] +
                    [(wq_sb, qT, th, f"q{th}") for th in range(4, 8)]):
                prework[g0 + 2 * i] = (
                    lambda w_sb=w_sb, dstT=dstT, th=th, nm=nm:
                    proj_group(psS, w_sb, dstT, th, nm))
            g0 += 16
            # batch-1 norm chains: squ+n2 per chain staged into quarters of
            # one f16 tile, then just TWO rsqrt instructions (k-chains and
            # q-chains), then the muls
            b1chains = [(kT, 2, False), (kT, 3, False),
                        (qT, 2, True), (qT, 3, True)]
            n2all = pe.tile([128, 4096], F16, tag="n2all")
            rqall = pe.tile([128, 4096], F16, tag="rqall")
            cstate = {}

            def sched_chain_sq(ci, dstT, tp, is_q):
                def go():
                    tfull = slice(tp * 1024, (tp + 1) * 1024)
                    squ = pc.tile([128, 1024], BF16, tag="squ")
                    nc.vector.tensor_mul(squ[:], dstT[:, tfull],
                                         dstT[:, tfull])
                    n2 = psS.tile([128, 2 * IB], F32, tag="sim",
                                  name=f"n2b1_{ci}")
                    for ti in range(2):
                        csl = slice(ti * 512, (ti + 1) * 512)
                        nc.tensor.matmul(n2[0:64, csl], ones_bf[0:64, 0:64],
                                         squ[0:64, csl], start=True,
                                         stop=True)
                        nc.tensor.matmul(n2[64:128, csl],
                                         ones_bf[64:128, 0:64],
                                         squ[64:128, csl], start=True,
                                         stop=True)
                    # stage to SBUF f16 so the psS slot frees immediately;
                    # fold the q-side gamma scale here so ONE rsqrt serves
                    # all four chains (no ACT-table ping-pong)
                    dsl = slice(ci * 1024, (ci + 1) * 1024)
                    if is_q:
                        nc.vector.tensor_scalar_mul(n2all[:, dsl], n2[:],
                                                    gq_sb[:])
                    else:
                        nc.vector.tensor_copy(n2all[:, dsl], n2[:])
                return go

            def sched_chain_rsqrts():
                nc.scalar.activation(rqall[:], n2all[:],
                                     AF.Abs_reciprocal_sqrt,
                                     bias=bias0[:], scale=1.0)

            def sched_chain_mul(ci, dstT, tp):
                def go():
                    tfull = slice(tp * 1024, (tp + 1) * 1024)
                    nc.vector.tensor_mul(
                        dstT[:, tfull], dstT[:, tfull],
                        rqall[:, ci * 1024:(ci + 1) * 1024])
                return go

            for ci, (dstT, tp, is_q) in enumerate(b1chains):
                prework[g0 + ci] = sched_chain_sq(ci, dstT, tp, is_q)
            g0 += 5
            prework[g0] = sched_chain_rsqrts
            g0 += 2
            for ci, (dstT, tp, is_q) in enumerate(b1chains):
                prework[g0 + ci] = sched_chain_mul(ci, dstT, tp)
            g0 += 4
            for th in range(4, 8):
                g0 = sched_vgroup(th, g0)

            sims = [None] * NG
            Ypair = [None] * (NG // 2)
            expair = [None] * (NG // 2)   # (kind, tile)
            outp = {}
            pending = []

            def is_A_pair(pp):
                return pp % 2 == 0

            def is_C_pair(pp):
                return False

            def ioff_of(blk):
                b, iq = blk // 4, blk % 4
                return b * N + iq * IB

            def emit_sim(g):
                blk, j = divmod(g, NJ)
                boff = (blk // 4) * N
                ioff = ioff_of(blk)
                jsl = slice(boff + j * 128, boff + (j + 1) * 128)
                isl = slice(ioff, ioff + IB)
                sim = psS.tile([128, 2 * IB], F32, tag="sim", name="sim")
                nc.tensor.matmul(sim[:, 0:IB], kT[0:64, jsl],
                                 qT[0:64, isl], start=True, stop=True)
                nc.tensor.matmul(sim[:, IB:2 * IB], kT[64:128, jsl],
                                 qT[64:128, isl], start=True, stop=True)
                sims[g] = sim

            def emit_stage1(g):
                pp = g // 2
                sim = sims[g]
                if g % 2 == 0:
                    Ypair[pp] = pY.tile([128, 2 * 2 * IB], F32, tag="Y",
                                        name="Y")
                dst = Ypair[pp][:, (g % 2) * 2 * IB:(g % 2 + 1) * 2 * IB]
                if is_A_pair(pp) or is_C_pair(pp):
                    nc.vector._custom_dve(op1, out=dst, in0=sim[:],
                                          s0=K5, s1=K3, imm2=K1)
                else:
                    nc.scalar.activation(dst, sim[:], AF.Tanh,
                                         bias=bias0[:],
                                         scale=1.0 / (SOFTCLAMP * LAM))
                sims[g] = None

            def emit_stage2(pp):
                Yp = Ypair[pp]
                if is_A_pair(pp):
                    ex = pxa.tile([128, 2 * 2 * IB], F32, tag="exA",
                                  name="exA")
                    nc.vector._custom_dve(op2, out=ex[:].bitcast(U32),
                                          in0=Yp[:], in1=b0t[:],
                                          s0=M_BIG, s1=B2, imm2=B1)
                    expair[pp] = ("A", ex)
                elif is_C_pair(pp):
                    # Yp holds op1's 2^23-scaled log2 value; exp it on ACT
                    ex = pxb.tile([128, 2 * 2 * IB], BF16, tag="exB",
                                  name="exB")
                    nc.scalar.activation(ex[:], Yp[:], AF.Exp,
                                         bias=bias_e[:],
                                         scale=float(np.log(2.0) / 2.0**23))
                    expair[pp] = ("B", ex)
                else:
                    ex = pxb.tile([128, 2 * 2 * IB], BF16, tag="exB",
                                  name="exB")
                    nc.scalar.activation(ex[:], Yp[:], AF.Exp,
                                         bias=bias_e[:], scale=6.25)
                    expair[pp] = ("B", ex)
                Ypair[pp] = None
                if dbg and pp <= 1:
                    which = "d_exA" if is_A_pair(pp) else "d_exB"
                    src = expair[pp][1][:]
                    if is_A_pair(pp):
                        src = src.bitcast(U32)
                    nc.sync.dma_start(out=dbg[which][:], in_=src)

            def ex_view(g, half):
                # [128, IB] moving-operand view of granule g's weights for
                # head `half` (0=A cols 0:IB, 1=B cols IB:2IB)
                kind, ex = expair[g // 2]
                off = (g % 2) * 2 * IB + half * IB
                if kind == "B":
                    return ex[:, off:off + IB]
                ap = ex[:, off:off + IB].bitcast(BF16)
                return ap.rearrange("p (n two) -> p n two", two=2)[:, :, 1]

            def emit_pv(g):
                blk, j = divmod(g, NJ)
                if j == 0:
                    outp[blk] = (
                        psO.tile([128, IB], F32, tag="outA", name="outA"),
                        psO.tile([128, IB], F32, tag="outB", name="outB"))
                outpA, outpB = outp[blk]
                jv = (blk // 4) * NJ + j
                st = (j == 0)
                sp = (j == NJ - 1)
                nc.tensor.matmul(outpA[:, 0:IB], v_sb[:, jv, 0:128],
                                 ex_view(g, 0), start=st, stop=sp)
                nc.tensor.matmul(outpB[:, 0:IB], v_sb[:, jv, 128:256],
                                 ex_view(g, 1), start=st, stop=sp)
                if g % 2 == 1:
                    expair[g // 2] = None

            def finish_exits(blk, last=False):
                outpA, outpB = outp.pop(blk)
                # denA = outpA[0:64], vA-out = outpA[64:128];
                # vB-out = outpB[0:64], denB = outpB[64:128].
                ra = pe.tile([128, IB], F32, tag="ra")
                rb = pe.tile([128, IB], F32, tag="rb")
                tmpA = pe.tile([128, IB], BF16, tag="tmpA")
                tmpB = pe.tile([128, IB], BF16, tag="tmpB")
                nc.vector.reciprocal_approx_fast(ra[:], outpA[:])
                nc.vector.tensor_copy(tmpA[64:128, :], outpA[64:128, :])
                nc.vector.reciprocal_approx_fast(rb[:], outpB[:])
                nc.vector.tensor_copy(tmpB[0:64, :], outpB[0:64, :])
                rs = pe.tile([128, IB], F32, tag="rs")
                nc.gpsimd.dma_start(out=rs[64:128, :], in_=ra[0:64, :])
                nc.gpsimd.dma_start(out=rs[0:64, :], in_=rb[64:128, :])

                attT = pe.tile([128, IB], BF16, tag="attT")

                def normalize():
                    # rows [0:64] = head B dims, [64:128] = head A dims
                    # (wo is host-reordered to match). Pool engine in
                    # steady state; the final block's drain uses the
                    # then-idle DVE (Pool is ~3x slower)
                    eng = nc.vector if last else nc.gpsimd
                    eng.tensor_mul(attT[0:64, :], tmpB[0:64, :],
                                   rs[0:64, :])
                    eng.tensor_mul(attT[64:128, :], tmpA[64:128, :],
                                   rs[64:128, :])

                pending.append(normalize)

                o_big = pe.tile([128, IB // 128, DIM], BF16, tag="obig")
                ioff = ioff_of(blk)

                def mk(tci):
                    def go():
                        po = psS.tile([128, DIM], F32, tag="sim", name="po")
                        tsl = slice(tci * 128, (tci + 1) * 128)
                        for ec in range(2):
                            esl = slice(ec * 512, (ec + 1) * 512)
                            nc.tensor.matmul(po[:, esl], attT[:, tsl],
                                             wo_sb[:, esl], start=True,
                                             stop=True)
                        if last and tci % 2 == 1:
                            nc.vector.tensor_copy(o_big[:, tci, :], po[:])
                        else:
                            nc.scalar.activation(o_big[:, tci, :], po[:],
                                                 AF.Copy)
                        r0 = ioff + tci * 128
                        nc.sync.dma_start(out=out[r0:r0 + 128, :],
                                          in_=o_big[:, tci, :])
                    return go

                for tci in range(IB // 128):
                    pending.append(mk(tci))

            # re-warm the PE across the lead-in chain latency gap
            warm2 = psS.tile([128, 2 * IB], F32, tag="sim", name="warm2")
            for _ in range(8):
                nc.tensor.matmul(warm2[:, 0:128], ones_bf[:], ident[:],
                                 start=True, stop=True,
                                 skip_group_check=True)
            emit_sim(0)
            emit_sim(1)
            emit_sim(2)
            for g in range(NG):
                emit_stage1(g)
                if g % 2 == 1:
                    emit_stage2(g // 2)
                if g in prework:
                    prework.pop(g)()
                elif pending and (len(pending) > 4 or g % 3 == 2):
                    pending.pop(0)()
                if g + 3 < NG:
                    emit_sim(g + 3)
                if g % 2 == 1 and g >= 3:
                    emit_pv(g - 3)
                    emit_pv(g - 2)
                if g % NJ == 6 and g > NJ:
                    finish_exits(g // NJ - 1)
            for gg in range(NG - 2, NG):
                emit_pv(gg)
            finish_exits(NBLK - 1, last=True)
            for fn in pending:
                fn()
            if dbg:
                nc.sync.dma_start(out=dbg["d_qT"][:], in_=qT[:])
                nc.sync.dma_start(out=dbg["d_kT"][:], in_=kT[:])
                nc.sync.dma_start(out=dbg["d_v"][:], in_=v_sb[:])


_NC = None


def _get_nc():
    global _NC
    if _NC is None:
        _NC = build_nc()
    return _NC


def _ensure_axon_hooks():
    try:
        import antenv.axon_hooks  # noqa: F401
        return
    except ImportError:
        pass
    import types
    hook = None
    try:
        if "/root/.axon_site" not in sys.path:
            sys.path.insert(0, "/root/.axon_site")
        from trn_agent_boot.trn_boot import _ntff_profile_via_ctypes
        hook = _ntff_profile_via_ctypes("/opt/axon/libaxon_pjrt.so")
    except Exception:
        hook = None
    m = types.ModuleType("antenv.axon_hooks")
    m.get_axon_ntff_profile_hook = lambda: hook
    sys.modules["antenv.axon_hooks"] = m


def kernel(tokens, norm_w, w_q, w_kv, w_out, q_gamma, k_gamma):
    tokens = np.asarray(tokens, np.float32)
    norm_w = np.asarray(norm_w, np.float32)
    w_q = np.asarray(w_q, np.float32)
    w_kv = np.asarray(w_kv, np.float32)
    w_out = np.asarray(w_out, np.float32)
    q_gamma = np.asarray(q_gamma, np.float32)
    k_gamma = np.asarray(k_gamma, np.float32)

    bf = ml_dtypes.bfloat16
    # host-side rmsnorm scale folded into the tokens (f32, exact)
    tok2 = tokens.reshape(T, DIM)
    s = 1.0 / np.sqrt((tok2 * tok2).mean(axis=1, keepdims=True) + RMS_EPS)
    tok_n = tok2 * s
    tok_bf = np.ascontiguousarray(tok_n.astype(bf).T)

    wq_f = norm_w[:, None] * w_q
    wkv_f = norm_w[:, None] * w_kv
    wk_f = wkv_f[:, :H * DH]
    wv_f = wkv_f[:, H * DH:]
    # combined q*k gamma scale (incl. both sqrt(DH) factors) and the custom
    # op's LAM pre-scale, applied on the q side
    g2_full = ((q_gamma + 1.0) * (k_gamma + 1.0) * float(DH)).reshape(H * DH)

    def _swz(w):
        return np.ascontiguousarray(
            w.astype(bf).reshape(DCH, 128, CD).transpose(1, 0, 2)
            .reshape(128, DCH * CD))

    in_maps = []
    for c in range(NCORES):
        cols = slice(c * CD, (c + 1) * CD)
        g2c = g2_full[c * CD:(c + 1) * CD] * LAM
        wo_c = w_out[cols, :]
        # attT rows are [head B dims, head A dims]
        wo_r = np.concatenate([wo_c[64:128, :], wo_c[0:64, :]], axis=0)
        in_maps.append({
            "tok": tok_bf,
            "wq": _swz(wq_f[:, cols]),
            "wk": _swz(wk_f[:, cols]),
            "wv": _swz(wv_f[:, cols]),
            "wo": np.ascontiguousarray(wo_r).astype(bf),
            "gq": np.ascontiguousarray(
                (1.0 / (g2c * g2c)).reshape(CD, 1), dtype=np.float32),
        })

    nc = _get_nc()
    trace = os.environ.get("KBENCH_TRACE") == "1"
    kwargs = {}
    if trace:
        _ensure_axon_hooks()
        import concourse.bass_utils as _bu
        _bu.upload_artifacts = lambda d: "local://" + d
        kwargs = {"trace": True,
                  "tmpdir": os.environ.get("KBENCH_TRACE_DIR") or None}
    res = run_bass_kernel_spmd(nc, in_maps, core_ids=list(range(NCORES)),
                               **kwargs)
    if res.exec_time_ns is not None:
        print(f"HW exec time: {res.exec_time_ns} ns")
    acc = np.zeros((T, DIM), np.float32)
    for i in range(NCORES):
        acc += res.results[i]["out"].astype(np.float32)
    return acc.reshape(B, N, DIM)


if __name__ == "__main__":
    rng = np.random.default_rng(0)
    inputs = {
        "tokens": rng.standard_normal((B, N, DIM), dtype=np.float32),
        "norm_w": np.ones((DIM,), np.float32),
        "w_q": rng.standard_normal((DIM, H * DH), dtype=np.float32) * 0.02,
        "w_kv": rng.standard_normal((DIM, 2 * H * DH), dtype=np.float32) * 0.02,
        "w_out": rng.standard_normal((H * DH, DIM), dtype=np.float32) * 0.02,
        "q_gamma": np.zeros((H, DH), np.float32),
        "k_gamma": np.zeros((H, DH), np.float32),
    }
    out = kernel(**inputs)
    print("out", out.shape, out.dtype, float(np.abs(out).max()))


# revision 75
# speedup vs baseline: 1.0061x; 1.0061x over previous
"""Softclamped multi-head attention (B=2, N=2048, DIM=1024, 16 heads x 64) on
8 TRN2 NeuronCores.

Sharding: tensor-parallel over heads - 2 heads per core. Each core computes its
heads' Q/K/V projections, attention, and a partial output projection; the 8
fp32 partials are summed on the host (the out-proj contraction dim is sharded),
so the device graph needs no collectives.

v2 structure (~287us -> target <200us):
  - rmsnorm is folded on the HOST: tokens are pre-scaled by rsqrt(mean(x^2))
    before the bf16 cast, killing the device-side sumsq matmuls, rsqrt and
    the v-epilogue multiply.
  - The softmax nonlinearity exp(6.25*tanh(sim/50)) is split across two
    engine paths per granule-PAIR:
      A-path (DVE): custom op TANH2Y computes Y = 2^23*log2(e)*6.25*
        tanh(s/50) as a deg-7 odd poly (input pre-scaled by lam so the
        sigma^7 coeff is exactly -1 => 3 scalar consts); custom op EXP2V
        turns Y into the f32 BIT PATTERN of 2^(y+0.5) via a magic-add
        round (n23 = (Y+M)-M at 2^23 granularity) + quadratic mantissa
        fit, written through the DVE's f32->u32 value conversion. The PV
        matmul reads the u32 tile's HIGH HALF-WORDS as a stride-2 bf16
        AP - bf16 bits are the f32 top 16 bits, so no engine ever runs
        a real exp.
      B-path (ACT): plain Tanh then Exp(6.25*t + 0.5*ln2) table ops,
        matching the A-path's sqrt(2) scale exactly.
    Both paths write per-PAIR [128, 2048] tiles; the pattern alternates
    pairs so DVE and ACT stay balanced.
  - the token DMA streams batch-major so all 8 batch-0 k/q projection
    groups pace per-sub-chunk in 8 PSUM banks during the load; batch-1
    projections + norm chains and all v-projection groups drain as
    scheduled prework items INSIDE phase D (batch-1 rsqrts batched
    back-to-back to bound ACT-table reloads).
  - v transposes are PE transposes into spare columns of the v-group's
    own PSUM slot + DVE copies (dma_start_transpose measured 1.2us per
    128x128 block of hwdge-queue time - too slow).
  - normalize muls ride the Pool (gpsimd) engine, which otherwise idles;
    po exits stay on ACT (moving any to DVE measured worse).

HW facts this relies on (probed on silicon in this session):
  - custom DVE f32->u32 output conversion is exact value truncation;
    magic-add rounding is IEEE RNE; full-tile Src1 works (only [P,1]
    broadcast Src1 is broken).
  - a matmul moving operand can be a stride-2 bf16 view of a u32 tile.
  - matmul dtype mixing 16/32-bit is rejected by walrus (hence the
    half-word trick instead of f32r).
  - shift/divide AluOps return 0 on the TRN2 DVE (hence the magic-add
    construction instead of exponent shifts).
  - gpsimd/Pool has no PSUM port; PSUM exits must use ACT/DVE.
"""

import os
os.environ.setdefault("JAX_PLATFORMS", "axon")
import sys
if "/opt/trn_rl_repo" not in sys.path:
    sys.path.insert(0, "/opt/trn_rl_repo")

import numpy as np
import ml_dtypes

import concourse.bass as bass  # noqa: F401
from concourse import bacc, mybir
import concourse.tile as tile
from concourse.bass_utils import run_bass_kernel_spmd

B, N, DIM = 2, 2048, 1024
H, DH = 16, 64
NCORES = 8
HPC = H // NCORES          # heads per core = 2
CD = HPC * DH              # per-core projection width = 128
T = B * N                  # 4096 tokens
DCH = DIM // 128           # 8 dim chunks
F32 = mybir.dt.float32
F16 = mybir.dt.float16
BF16 = mybir.dt.bfloat16
U32 = mybir.dt.uint32
AF = mybir.ActivationFunctionType
IB = 512                   # attention i-block (queries per phase-D block)
NBLK = T // IB             # 8 blocks
NJ = N // 128              # 16 key chunks per batch
NG = NBLK * NJ             # 128 granules, 64 pairs

SOFTCLAMP = 50.0
RMS_EPS = 1e-6

# ---------------------------------------------------------------------------
# Custom-op numerics (see fit_ops.py for derivation + numpy validation).
# ---------------------------------------------------------------------------
AMP = 2.0**23 * np.log2(np.e) * 6.25
TA, TB_, TC, TD = 0.99819183, -0.31795733, 0.09607557, -0.01496778
_a1 = AMP * TA / 50.0
_a3 = AMP * TB_ / 50.0**3
_a5 = AMP * TC / 50.0**5
_a7 = AMP * TD / 50.0**7
LAM = float((-_a7) ** (1.0 / 7.0))   # sigma = LAM*s ; sigma^7 coeff == -1
K5 = float(np.float32(_a5 / LAM**5))
K3 = float(np.float32(_a3 / LAM**3))
K1 = float(np.float32(_a1 / LAM))

Q0, Q1, Q2 = 1.41569374, 0.9921173, 0.3220771   # 2^(f+0.5) quad minimax
M_BIG = float(np.float32(1.5 * 2.0**46))
# +2^15 nudges the bf16 high-halfword truncation to round-to-nearest
B0 = float(np.float32(2.0**23 * (126.0 + Q0) + 32768.0))
B1 = float(np.float32(Q1))
B2 = float(np.float32(Q2 / 2.0**23))
EXP_B_BIAS = float(0.5 * np.log(2.0))   # B-path: w = exp(6.25*t + 0.5*ln2)

_OPS = None


def _register_ops():
    global _OPS
    if _OPS is not None:
        return _OPS
    import concourse.dve_ops as dve_ops
    from concourse.dve_spec import (Spec, Src0, Src1, C0, C1, C2, lower,
                                    _has_src1)
    from concourse.dve_uop import DveOpSpec

    def mk(name, body, ref):
        if name in dve_ops._SUB_OPCODE_FOR_NAME:
            return next(o for o in dve_ops.OPS if o.name == name)
        spec = Spec(body=body, reference=ref)
        row = dve_ops._CUSTOM_DVE_ROW_BASE + len(dve_ops.OPS)
        assert row < 0x20
        dve_ops._SUB_OPCODE_FOR_NAME[name] = row
        shas = {}
        for ver in ("v3", "v4"):
            uops = lower(spec, ver=ver)
            shas[ver] = DveOpSpec(name=name, opcode=row, uops=uops,
                                  rd1_en=_has_src1(spec)).sha(ver)
        op = dve_ops.DveOp(name, spec, subdim=False, uops_sha=shas)
        dve_ops.OPS.append(op)
        dve_ops.CUSTOM_DVE_SPECS[name] = op.spec
        return op

    u = Src0 * Src0
    body1 = ((((C0 - u) * u + C1) * u) + C2) * Src0

    def ref1(in0, in1, s0, s1, imm2):
        x = in0.astype(np.float32)
        uu = x * x
        return ((((s0 - uu) * uu + s1) * uu) + imm2) * x

    n23 = (Src0 + C0) - C0
    Fv = Src0 - n23
    body2 = (((Fv * C1) + C2) * Fv + Src1) + n23

    def ref2(in0, in1, s0, s1, imm2):
        Y = in0.astype(np.float32)
        nn = ((Y + s0).astype(np.float32) - s0).astype(np.float32)
        Ff = (Y - nn).astype(np.float32)
        t = ((Ff * s1).astype(np.float32) + imm2).astype(np.float32)
        t = ((t * Ff).astype(np.float32) + in1).astype(np.float32)
        return (t + nn).astype(np.float32)

    _OPS = (mk("TANH2Y_ANT", body1, ref1), mk("EXP2V_ANT", body2, ref2))
    return _OPS


def build_nc(debug_outs=False):
    op1, op2 = _register_ops()
    nc = bacc.Bacc("TRN2", target_bir_lowering=False, debug=False,
                   num_devices=NCORES)
    tok = nc.declare_dram_parameter("tok", [DIM, T], BF16, isOutput=False)
    wq = nc.declare_dram_parameter("wq", [128, DCH * CD], BF16, isOutput=False)
    wk = nc.declare_dram_parameter("wk", [128, DCH * CD], BF16, isOutput=False)
    wv = nc.declare_dram_parameter("wv", [128, DCH * CD], BF16, isOutput=False)
    wo = nc.declare_dram_parameter("wo", [CD, DIM], BF16, isOutput=False)
    # per-partition scale for the q-norm rsqrt: 1/(g2*LAM)^2
    gq = nc.declare_dram_parameter("gq", [CD, 1], F32, isOutput=False)
    out = nc.declare_dram_parameter("out", [T, DIM], BF16, isOutput=True)
    dbg = None
    if debug_outs:
        dbg = {
            "d_qT": nc.declare_dram_parameter("d_qT", [128, T], BF16, True),
            "d_kT": nc.declare_dram_parameter("d_kT", [128, T], BF16, True),
            "d_v": nc.declare_dram_parameter("d_v", [128, T // 128, 256],
                                             BF16, True),
            "d_exA": nc.declare_dram_parameter("d_exA", [128, 2048], U32, True),
            "d_exB": nc.declare_dram_parameter("d_exB", [128, 2048], BF16, True),
        }

    with tile.TileContext(nc) as tc:
        _emit(nc, tc, op1, op2, tok, wq, wk, wv, wo, gq, out, dbg)
    nc.compile()
    return nc


def _emit(nc, tc, op1, op2, tok, wq, wk, wv, wo, gq, out, dbg=None):
    with tc.tile_pool(name="const", bufs=1) as const, \
         tc.tile_pool(name="core", bufs=1) as core:

        # ---- constants / weights ----
        from concourse.masks import make_identity
        ones_bf = const.tile([128, 128], BF16, tag="ones")
        nc.vector.memset(ones_bf[:], 1.0)
        ident = const.tile([128, 128], BF16, tag="ident")
        make_identity(nc, ident[:])
        bias0 = const.tile([128, 1], F32, tag="bias0")
        nc.vector.memset(bias0[:], 0.0)
        bias_e = const.tile([128, 1], F32, tag="bias_e")
        nc.vector.memset(bias_e[:], EXP_B_BIAS)
        b0t = const.tile([128, 2 * 2 * IB], F32, tag="b0t")
        nc.vector.memset(b0t[:], B0)
        gq_sb = const.tile([128, 1], F32, tag="gq")
        nc.scalar.dma_start(out=gq_sb[:], in_=gq[:])
        wq_sb = const.tile([128, DCH, CD], BF16, tag="wq")
        wk_sb = const.tile([128, DCH, CD], BF16, tag="wk")
        wv_sb = const.tile([128, DCH, CD], BF16, tag="wv")
        # wk/wq ride the sync queue AHEAD of the token stream so the first
        # projection matmuls aren't starved behind 8 MB of tokens
        for w_dram, w_sb in ((wk, wk_sb), (wq, wq_sb)):
            nc.sync.dma_start(out=w_sb[:],
                              in_=w_dram.rearrange("p (c m) -> p c m",
                                                   c=DCH))
        nc.scalar.dma_start(out=wv_sb[:],
                            in_=wv.rearrange("p (c m) -> p c m", c=DCH))
        wo_sb = const.tile([128, DIM], BF16, tag="wo")
        nc.scalar.dma_start(out=wo_sb[:], in_=wo[:])

        # persistent tensors
        qT = core.tile([128, T], BF16, tag="qT")
        kT = core.tile([128, T], BF16, tag="kT")
        vT = core.tile([128, T], BF16, tag="vT")
        # augmented v, per 128-token chunk: [onesA(64)|vA(64)|vB(64)|onesB(64)]
        v_sb = core.tile([128, T // 128, 256], BF16, tag="v")
        nc.vector.memset(v_sb[:, :, 0:64], 1.0)
        nc.vector.memset(v_sb[:, :, 192:256], 1.0)

        tok_ch = [core.tile([128, T], BF16, tag=f"tok{ch}", name=f"tok{ch}")
                  for ch in range(DCH)]
        # batch-major sub-chunk stream: each (b, ch) arrival unleashes the
        # 8 resident projection groups' ch-matmuls for that batch, so the
        # whole q+k projection pass is DMA-paced in 8 PSUM banks.
        for b in range(B):
            nsl = slice(b * N, (b + 1) * N)
            for ch in range(DCH):
                # alternate hwdge queues: each runs at ~83% descriptor
                # efficiency, so two in parallel approach the full HBM rate
                q = nc.sync if ch % 2 == 0 else nc.scalar
                q.dma_start(out=tok_ch[ch][:, nsl],
                            in_=tok[ch * 128:(ch + 1) * 128, nsl])

        # ------------------------------------------------------------------
        # projection-group + head-norm emitters (psum pool passed per phase)
        # ------------------------------------------------------------------
        def proj_group(ps_pool, w_sb, dstT, th, name, copy_dve=False):
            tsl = slice(th * 512, (th + 1) * 512)
            pq = ps_pool.tile([128, 2 * IB], F32, tag="sim", name=name)
            for ch in range(DCH):
                nc.tensor.matmul(pq[:, 0:512], w_sb[:, ch, :],
                                 tok_ch[ch][:, tsl], start=(ch == 0),
                                 stop=True, skip_group_check=True)
            if copy_dve:
                nc.vector.tensor_copy(dstT[:, tsl], pq[:, 0:512])
            else:
                nc.scalar.activation(dstT[:, tsl], pq[:, 0:512], AF.Copy)
            return pq

        def norm_chains_batched(ps_pool, pc_pool, specs, psum_shape=2 * IB,
                                tag="sim"):
            # specs: list of (dstT, tp, is_q, name). All squ muls first,
            # then all n2 matmuls, then rsqrts, then the in-place muls —
            # each engine's FIFO pipelines instead of ping-ponging.
            # n2[p, t] = sum of squares over the head owning partition p.
            sq_tiles, rq_tiles, n2s = [], [], []
            for dstT, tp, is_q, name in specs:
                tfull = slice(tp * 1024, (tp + 1) * 1024)
                squ = pc_pool.tile([128, 1024], BF16, tag="squ")
                nc.vector.tensor_mul(squ[:], dstT[:, tfull], dstT[:, tfull])
                sq_tiles.append(squ)
            for squ, (dstT, tp, is_q, name) in zip(sq_tiles, specs):
                pair = []
                for ti in range(2):
                    csl = slice(ti * 512, (ti + 1) * 512)
                    n2 = ps_pool.tile([128, psum_shape], F32, tag=tag,
                                      name=f"{name}_{ti}")
                    nc.tensor.matmul(n2[0:64, 0:512], ones_bf[0:64, 0:64],
                                     squ[0:64, csl], start=True, stop=True)
                    nc.tensor.matmul(n2[64:128, 0:512], ones_bf[64:128, 0:64],
                                     squ[64:128, csl], start=True, stop=True)
                    pair.append(n2)
                n2s.append(pair)
            for pair, (dstT, tp, is_q, name) in zip(n2s, specs):
                rq = pc_pool.tile([128, 1024], F16, tag="rq")
                sc = gq_sb[:] if is_q else 1.0
                for ti in range(2):
                    csl = slice(ti * 512, (ti + 1) * 512)
                    nc.scalar.activation(rq[:, csl], pair[ti][:, 0:512],
                                         AF.Abs_reciprocal_sqrt,
                                         bias=bias0[:], scale=sc)
                rq_tiles.append(rq)
            for rq, (dstT, tp, is_q, name) in zip(rq_tiles, specs):
                tfull = slice(tp * 1024, (tp + 1) * 1024)
                nc.vector.tensor_mul(dstT[:, tfull], dstT[:, tfull], rq[:])

        def vproj_mm(ps_pool, th, name):
            return proj_group(ps_pool, wv_sb, vT, th, name)

        def vtr(pq, i, tv):
            # transposes ride unused columns of the group's own pq slot
            ptr = pq[:, 512 + i * 64: 512 + (i + 1) * 64].bitcast(BF16)
            nc.tensor.transpose(ptr,
                                vT[:, tv * 128:(tv + 1) * 128], ident[:])
            nc.vector.tensor_copy(v_sb[:, tv, 64:192].bitcast(U32),
                                  ptr.bitcast(U32))

        # ---- lead-in: ALL q/k projections + head norms, chunk-major with
        # 8 resident accumulation banks per batch pass, so PE work is paced
        # by the token DMA stream. Keeps the Abs_reciprocal_sqrt ACT-table
        # state out of phase D (tanh/exp/copy share one table set). ----
        # ---- lead-in: BATCH-0 q/k projections + head norms only. Batch-1
        # projections and norms drain inside phase D (its first 4 blocks
        # touch only batch 0), with their rsqrts batched back-to-back so
        # the tanh/exp ACT-table is reloaded exactly twice. ----
        with tc.tile_pool(name="psc", bufs=8, space="PSUM") as psc, \
             tc.tile_pool(name="pc0", bufs=2) as pc0:
            # PE HAM warm-up: dummy matmuls before the first token
            # sub-chunk lands, so the projection stream runs at 2.4 GHz
            warm = psc.tile([128, 512], F32, tag="pp", name="warm")
            for _ in range(30):
                nc.tensor.matmul(warm[:, 0:128], ones_bf[:], ident[:],
                                 start=True, stop=True,
                                 skip_group_check=True)
            pqk = {}
            for th in range(4):
                pqk["k", th] = psc.tile([128, 512], F32, tag="pp",
                                        name=f"pk{th}")
                pqk["q", th] = psc.tile([128, 512], F32, tag="pp",
                                        name=f"pq{th}")
            for ch in range(DCH):
                for th in range(4):
                    tsl = slice(th * 512, (th + 1) * 512)
                    nc.tensor.matmul(pqk["k", th][:], wk_sb[:, ch, :],
                                     tok_ch[ch][:, tsl], start=(ch == 0),
                                     stop=True, skip_group_check=True)
                    nc.tensor.matmul(pqk["q", th][:], wq_sb[:, ch, :],
                                     tok_ch[ch][:, tsl], start=(ch == 0),
                                     stop=True, skip_group_check=True)
            # k copies on ACT, q copies on the otherwise-idle DVE so the
            # two copy streams drain in parallel ahead of the chains
            for th in range(4):
                tsl = slice(th * 512, (th + 1) * 512)
                nc.scalar.activation(kT[:, tsl], pqk["k", th][:], AF.Copy)
                nc.vector.tensor_copy(qT[:, tsl], pqk["q", th][:])
            norm_chains_batched(
                psc, pc0,
                [(kT, 0, False, "kn0"), (qT, 0, True, "qn0"),
                 (kT, 1, False, "kn1"), (qT, 1, True, "qn1")],
                psum_shape=512, tag="pp")
            # prefetch the tanh+exp table set before phase D (Tanh then Exp
            # back-to-back forces a set containing both: exp_and_others)
            scr = pc0.tile([128, 16], F32, tag="scr")
            nc.scalar.activation(scr[:], ones_bf[:, 0:16], AF.Tanh,
                                 bias=bias0[:], scale=1.0)
            nc.scalar.activation(scr[:], ones_bf[:, 0:16], AF.Exp,
                                 bias=bias0[:], scale=1.0)

        # ---- phase D: attention, software-pipelined ----
        with tc.tile_pool(name="psO", bufs=1, space="PSUM") as psO, \
             tc.tile_pool(name="psS", bufs=3, space="PSUM") as psS, \
             tc.tile_pool(name="pc", bufs=2) as pc, \
             tc.tile_pool(name="pY", bufs=2) as pY, \
             tc.tile_pool(name="pxa", bufs=2) as pxa, \
             tc.tile_pool(name="pxb", bufs=2) as pxb, \
             tc.tile_pool(name="pe", bufs=1) as pe:

            # prework schedule: granule index -> emission item. v-groups for
            # batch 0 first (PV deadline g=5..20), then batch-1 q/k
            # projections + norms (sim deadline g=58), then batch-1 v
            # (PV deadline g=69+). Transpose/copy pairs trail their group's
            # matmuls so the DVE copy never head-blocks on a fresh
            # transpose.
            prework = {}

            def sched_vgroup(th, g0):
                # mm-group at g0, all 4 transpose+copy pairs at g0+1 (the pq
                # slot is recycled by the sim allocated 2 granules later, so
                # every read must be emitted by then)
                state = {}

                def mm():
                    state["pq"] = vproj_mm(psS, th, f"v{th}")

                def trs():
                    for i in range(4):
                        vtr(state["pq"], i, th * 4 + i)
                prework[g0] = mm
                prework[g0 + 1] = trs
                return g0 + 2

            g0 = 0
            for th in range(4):
                g0 = sched_vgroup(th, g0)
            # batch-1 k/q projection groups, 1 item per 2 granules
            for i, (w_sb, dstT, th, nm) in enumerate(
                    [(wk_sb, kT, th, f"k{th}") for th in range(4, 8)] +
                    [(wq_sb, qT, th, f"q{th}") for th in range(4, 8)]):
                prework[g0 + 2 * i] = (
                    lambda w_sb=w_sb, dstT=dstT, th=th, nm=nm:
                    proj_group(psS, w_sb, dstT, th, nm))
            g0 += 16
            # batch-1 norm chains: squ+n2 per chain staged into quarters of
            # one f16 tile, then just TWO rsqrt instructions (k-chains and
            # q-chains), then the muls
            b1chains = [(kT, 2, False), (kT, 3, False),
                        (qT, 2, True), (qT, 3, True)]
            n2all = pe.tile([128, 4096], F16, tag="n2all")
            rqall = pe.tile([128, 4096], F16, tag="rqall")
            cstate = {}

            def sched_chain_sq(ci, dstT, tp, is_q):
                def go():
                    tfull = slice(tp * 1024, (tp + 1) * 1024)
                    squ = pc.tile([128, 1024], BF16, tag="squ")
                    nc.vector.tensor_mul(squ[:], dstT[:, tfull],
                                         dstT[:, tfull])
                    n2 = psS.tile([128, 2 * IB], F32, tag="sim",
                                  name=f"n2b1_{ci}")
                    for ti in range(2):
                        csl = slice(ti * 512, (ti + 1) * 512)
                        nc.tensor.matmul(n2[0:64, csl], ones_bf[0:64, 0:64],
                                         squ[0:64, csl], start=True,
                                         stop=True)
                        nc.tensor.matmul(n2[64:128, csl],
                                         ones_bf[64:128, 0:64],
                                         squ[64:128, csl], start=True,
                                         stop=True)
                    # stage to SBUF f16 so the psS slot frees immediately;
                    # fold the q-side gamma scale here so ONE rsqrt serves
                    # all four chains (no ACT-table ping-pong)
                    dsl = slice(ci * 1024, (ci + 1) * 1024)
                    if is_q:
                        nc.vector.tensor_scalar_mul(n2all[:, dsl], n2[:],
                                                    gq_sb[:])
                    else:
                        nc.vector.tensor_copy(n2all[:, dsl], n2[:])
                return go

            def sched_chain_rsqrts():
                nc.scalar.activation(rqall[:], n2all[:],
                                     AF.Abs_reciprocal_sqrt,
                                     bias=bias0[:], scale=1.0)

            def sched_chain_mul(ci, dstT, tp):
                def go():
                    tfull = slice(tp * 1024, (tp + 1) * 1024)
                    nc.vector.tensor_mul(
                        dstT[:, tfull], dstT[:, tfull],
                        rqall[:, ci * 1024:(ci + 1) * 1024])
                return go

            for ci, (dstT, tp, is_q) in enumerate(b1chains):
                prework[g0 + ci] = sched_chain_sq(ci, dstT, tp, is_q)
            g0 += 5
            prework[g0] = sched_chain_rsqrts
            g0 += 2
            for ci, (dstT, tp, is_q) in enumerate(b1chains):
                prework[g0 + ci] = sched_chain_mul(ci, dstT, tp)
            g0 += 4
            for th in range(4, 8):
                g0 = sched_vgroup(th, g0)

            sims = [None] * NG
            Ypair = [None] * (NG // 2)
            expair = [None] * (NG // 2)   # (kind, tile)
            outp = {}
            pending = []

            def is_A_pair(pp):
                return pp % 2 == 0

            def is_C_pair(pp):
                return False

            def ioff_of(blk):
                b, iq = blk // 4, blk % 4
                return b * N + iq * IB

            def emit_sim(g):
                blk, j = divmod(g, NJ)
                boff = (blk // 4) * N
                ioff = ioff_of(blk)
                jsl = slice(boff + j * 128, boff + (j + 1) * 128)
                isl = slice(ioff, ioff + IB)
                sim = psS.tile([128, 2 * IB], F32, tag="sim", name="sim")
                nc.tensor.matmul(sim[:, 0:IB], kT[0:64, jsl],
                                 qT[0:64, isl], start=True, stop=True)
                nc.tensor.matmul(sim[:, IB:2 * IB], kT[64:128, jsl],
                                 qT[64:128, isl], start=True, stop=True)
                sims[g] = sim

            def emit_stage1(g):
                pp = g // 2
                sim = sims[g]
                if g % 2 == 0:
                    Ypair[pp] = pY.tile([128, 2 * 2 * IB], F32, tag="Y",
                                        name="Y")
                dst = Ypair[pp][:, (g % 2) * 2 * IB:(g % 2 + 1) * 2 * IB]
                if is_A_pair(pp) or is_C_pair(pp):
                    nc.vector._custom_dve(op1, out=dst, in0=sim[:],
                                          s0=K5, s1=K3, imm2=K1)
                else:
                    nc.scalar.activation(dst, sim[:], AF.Tanh,
                                         bias=bias0[:],
                                         scale=1.0 / (SOFTCLAMP * LAM))
                sims[g] = None

            def emit_stage2(pp):
                Yp = Ypair[pp]
                if is_A_pair(pp):
                    ex = pxa.tile([128, 2 * 2 * IB], F32, tag="exA",
                                  name="exA")
                    nc.vector._custom_dve(op2, out=ex[:].bitcast(U32),
                                          in0=Yp[:], in1=b0t[:],
                                          s0=M_BIG, s1=B2, imm2=B1)
                    expair[pp] = ("A", ex)
                elif is_C_pair(pp):
                    # Yp holds op1's 2^23-scaled log2 value; exp it on ACT
                    ex = pxb.tile([128, 2 * 2 * IB], BF16, tag="exB",
                                  name="exB")
                    nc.scalar.activation(ex[:], Yp[:], AF.Exp,
                                         bias=bias_e[:],
                                         scale=float(np.log(2.0) / 2.0**23))
                    expair[pp] = ("B", ex)
                else:
                    ex = pxb.tile([128, 2 * 2 * IB], BF16, tag="exB",
                                  name="exB")
                    nc.scalar.activation(ex[:], Yp[:], AF.Exp,
                                         bias=bias_e[:], scale=6.25)
                    expair[pp] = ("B", ex)
                Ypair[pp] = None
                if dbg and pp <= 1:
                    which = "d_exA" if is_A_pair(pp) else "d_exB"
                    src = expair[pp][1][:]
                    if is_A_pair(pp):
                        src = src.bitcast(U32)
                    nc.sync.dma_start(out=dbg[which][:], in_=src)

            def ex_view(g, half):
                # [128, IB] moving-operand view of granule g's weights for
                # head `half` (0=A cols 0:IB, 1=B cols IB:2IB)
                kind, ex = expair[g // 2]
                off = (g % 2) * 2 * IB + half * IB
                if kind == "B":
                    return ex[:, off:off + IB]
                ap = ex[:, off:off + IB].bitcast(BF16)
                return ap.rearrange("p (n two) -> p n two", two=2)[:, :, 1]

            def emit_pv(g):
                blk, j = divmod(g, NJ)
                if j == 0:
                    outp[blk] = (
                        psO.tile([128, IB], F32, tag="outA", name="outA"),
                        psO.tile([128, IB], F32, tag="outB", name="outB"))
                outpA, outpB = outp[blk]
                jv = (blk // 4) * NJ + j
                st = (j == 0)
                sp = (j == NJ - 1)
                nc.tensor.matmul(outpA[:, 0:IB], v_sb[:, jv, 0:128],
                                 ex_view(g, 0), start=st, stop=sp)
                nc.tensor.matmul(outpB[:, 0:IB], v_sb[:, jv, 128:256],
                                 ex_view(g, 1), start=st, stop=sp)
                if g % 2 == 1:
                    expair[g // 2] = None

            def finish_exits(blk):
                outpA, outpB = outp.pop(blk)
                # denA = outpA[0:64], vA-out = outpA[64:128];
                # vB-out = outpB[0:64], denB = outpB[64:128].
                ra = pe.tile([128, IB], F32, tag="ra")
                rb = pe.tile([128, IB], F32, tag="rb")
                tmpA = pe.tile([128, IB], BF16, tag="tmpA")
                tmpB = pe.tile([128, IB], BF16, tag="tmpB")
                nc.vector.reciprocal_approx_fast(ra[:], outpA[:])
                nc.vector.tensor_copy(tmpA[64:128, :], outpA[64:128, :])
                nc.vector.reciprocal_approx_fast(rb[:], outpB[:])
                nc.vector.tensor_copy(tmpB[0:64, :], outpB[0:64, :])
                rs = pe.tile([128, IB], F32, tag="rs")
                nc.gpsimd.dma_start(out=rs[64:128, :], in_=ra[0:64, :])
                nc.gpsimd.dma_start(out=rs[0:64, :], in_=rb[64:128, :])

                attT = pe.tile([128, IB], BF16, tag="attT")

                def normalize():
                    # rows [0:64] = head B dims, [64:128] = head A dims
                    # (wo is host-reordered to match); Pool engine
                    nc.gpsimd.tensor_mul(attT[0:64, :], tmpB[0:64, :],
                                         rs[0:64, :])
                    nc.gpsimd.tensor_mul(attT[64:128, :], tmpA[64:128, :],
                                         rs[64:128, :])

                pending.append(normalize)

                o_big = pe.tile([128, IB // 128, DIM], BF16, tag="obig")
                ioff = ioff_of(blk)

                def mk(tci):
                    def go():
                        po = psS.tile([128, DIM], F32, tag="sim", name="po")
                        tsl = slice(tci * 128, (tci + 1) * 128)
                        for ec in range(2):
                            esl = slice(ec * 512, (ec + 1) * 512)
                            nc.tensor.matmul(po[:, esl], attT[:, tsl],
                                             wo_sb[:, esl], start=True,
                                             stop=True)
                        nc.scalar.activation(o_big[:, tci, :], po[:],
                                             AF.Copy)
                        r0 = ioff + tci * 128
                        nc.sync.dma_start(out=out[r0:r0 + 128, :],
                                          in_=o_big[:, tci, :])
                    return go

                for tci in range(IB // 128):
                    pending.append(mk(tci))

            # re-warm the PE across the lead-in chain latency gap
            warm2 = psS.tile([128, 2 * IB], F32, tag="sim", name="warm2")
            for _ in range(8):
                nc.tensor.matmul(warm2[:, 0:128], ones_bf[:], ident[:],
                                 start=True, stop=True,
                                 skip_group_check=True)
            emit_sim(0)
            emit_sim(1)
            emit_sim(2)
            for g in range(NG):
                emit_stage1(g)
                if g % 2 == 1:
                    emit_stage2(g // 2)
                if g in prework:
                    prework.pop(g)()
                elif pending and (len(pending) > 4 or g % 3 == 2):
                    pending.pop(0)()
                if g + 3 < NG:
                    emit_sim(g + 3)
                if g % 2 == 1 and g >= 3:
                    emit_pv(g - 3)
                    emit_pv(g - 2)
                if g % NJ == 6 and g > NJ:
                    finish_exits(g // NJ - 1)
            for gg in range(NG - 2, NG):
                emit_pv(gg)
            finish_exits(NBLK - 1)
            for fn in pending:
                fn()
            if dbg:
                nc.sync.dma_start(out=dbg["d_qT"][:], in_=qT[:])
                nc.sync.dma_start(out=dbg["d_kT"][:], in_=kT[:])
                nc.sync.dma_start(out=dbg["d_v"][:], in_=v_sb[:])


_NC = None


def _get_nc():
    global _NC
    if _NC is None:
        _NC = build_nc()
    return _NC


def _ensure_axon_hooks():
    try:
        import antenv.axon_hooks  # noqa: F401
        return
    except ImportError:
        pass
    import types
    hook = None
    try:
        if "/root/.axon_site" not in sys.path:
            sys.path.insert(0, "/root/.axon_site")
        from trn_agent_boot.trn_boot import _ntff_profile_via_ctypes
        hook = _ntff_profile_via_ctypes("/opt/axon/libaxon_pjrt.so")
    except Exception:
        hook = None
    m = types.ModuleType("antenv.axon_hooks")
    m.get_axon_ntff_profile_hook = lambda: hook
    sys.modules["antenv.axon_hooks"] = m


def kernel(tokens, norm_w, w_q, w_kv, w_out, q_gamma, k_gamma):
    tokens = np.asarray(tokens, np.float32)
    norm_w = np.asarray(norm_w, np.float32)
    w_q = np.asarray(w_q, np.float32)
    w_kv = np.asarray(w_kv, np.float32)
    w_out = np.asarray(w_out, np.float32)
    q_gamma = np.asarray(q_gamma, np.float32)
    k_gamma = np.asarray(k_gamma, np.float32)

    bf = ml_dtypes.bfloat16
    # host-side rmsnorm scale folded into the tokens (f32, exact)
    tok2 = tokens.reshape(T, DIM)
    s = 1.0 / np.sqrt((tok2 * tok2).mean(axis=1, keepdims=True) + RMS_EPS)
    tok_n = tok2 * s
    tok_bf = np.ascontiguousarray(tok_n.astype(bf).T)

    wq_f = norm_w[:, None] * w_q
    wkv_f = norm_w[:, None] * w_kv
    wk_f = wkv_f[:, :H * DH]
    wv_f = wkv_f[:, H * DH:]
    # combined q*k gamma scale (incl. both sqrt(DH) factors) and the custom
    # op's LAM pre-scale, applied on the q side
    g2_full = ((q_gamma + 1.0) * (k_gamma + 1.0) * float(DH)).reshape(H * DH)

    def _swz(w):
        return np.ascontiguousarray(
            w.astype(bf).reshape(DCH, 128, CD).transpose(1, 0, 2)
            .reshape(128, DCH * CD))

    in_maps = []
    for c in range(NCORES):
        cols = slice(c * CD, (c + 1) * CD)
        g2c = g2_full[c * CD:(c + 1) * CD] * LAM
        wo_c = w_out[cols, :]
        # attT rows are [head B dims, head A dims]
        wo_r = np.concatenate([wo_c[64:128, :], wo_c[0:64, :]], axis=0)
        in_maps.append({
            "tok": tok_bf,
            "wq": _swz(wq_f[:, cols]),
            "wk": _swz(wk_f[:, cols]),
            "wv": _swz(wv_f[:, cols]),
            "wo": np.ascontiguousarray(wo_r).astype(bf),
            "gq": np.ascontiguousarray(
                (1.0 / (g2c * g2c)).reshape(CD, 1), dtype=np.float32),
        })

    nc = _get_nc()
    trace = os.environ.get("KBENCH_TRACE") == "1"
    kwargs = {}
    if trace:
        _ensure_axon_hooks()
        import concourse.bass_utils as _bu
        _bu.upload_artifacts = lambda d: "local://" + d
        kwargs = {"trace": True,
                  "tmpdir": os.environ.get("KBENCH_TRACE_DIR") or None}
    res = run_bass_kernel_spmd(nc, in_maps, core_ids=list(range(NCORES)),
                               **kwargs)
    if res.exec_time_ns is not None:
        print(f"HW exec time: {res.exec_time_ns} ns")
    acc = np.zeros((T, DIM), np.float32)
    for i in range(NCORES):
        acc += res.results[i]["out"].astype(np.float32)
    return acc.reshape(B, N, DIM)


if __name__ == "__main__":
    rng = np.random.default_rng(0)
    inputs = {
        "tokens": rng.standard_normal((B, N, DIM), dtype=np.float32),
        "norm_w": np.ones((DIM,), np.float32),
        "w_q": rng.standard_normal((DIM, H * DH), dtype=np.float32) * 0.02,
        "w_kv": rng.standard_normal((DIM, 2 * H * DH), dtype=np.float32) * 0.02,
        "w_out": rng.standard_normal((H * DH, DIM), dtype=np.float32) * 0.02,
        "q_gamma": np.zeros((H, DH), np.float32),
        "k_gamma": np.zeros((H, DH), np.float32),
    }
    out = kernel(**inputs)
    print("out", out.shape, out.dtype, float(np.abs(out).max()))


# revision 76
# speedup vs baseline: 1.0121x; 1.0059x over previous
"""Softclamped multi-head attention (B=2, N=2048, DIM=1024, 16 heads x 64) on
8 TRN2 NeuronCores.

Sharding: tensor-parallel over heads - 2 heads per core. Each core computes its
heads' Q/K/V projections, attention, and a partial output projection; the 8
fp32 partials are summed on the host (the out-proj contraction dim is sharded),
so the device graph needs no collectives.

v2 structure (~287us -> target <200us):
  - rmsnorm is folded on the HOST: tokens are pre-scaled by rsqrt(mean(x^2))
    before the bf16 cast, killing the device-side sumsq matmuls, rsqrt and
    the v-epilogue multiply.
  - The softmax nonlinearity exp(6.25*tanh(sim/50)) is split across two
    engine paths per granule-PAIR:
      A-path (DVE): custom op TANH2Y computes Y = 2^23*log2(e)*6.25*
        tanh(s/50) as a deg-7 odd poly (input pre-scaled by lam so the
        sigma^7 coeff is exactly -1 => 3 scalar consts); custom op EXP2V
        turns Y into the f32 BIT PATTERN of 2^(y+0.5) via a magic-add
        round (n23 = (Y+M)-M at 2^23 granularity) + quadratic mantissa
        fit, written through the DVE's f32->u32 value conversion. The PV
        matmul reads the u32 tile's HIGH HALF-WORDS as a stride-2 bf16
        AP - bf16 bits are the f32 top 16 bits, so no engine ever runs
        a real exp.
      B-path (ACT): plain Tanh then Exp(6.25*t + 0.5*ln2) table ops,
        matching the A-path's sqrt(2) scale exactly.
    Both paths write per-PAIR [128, 2048] tiles; the pattern alternates
    pairs so DVE and ACT stay balanced.
  - the token DMA streams batch-major so all 8 batch-0 k/q projection
    groups pace per-sub-chunk in 8 PSUM banks during the load; batch-1
    projections + norm chains and all v-projection groups drain as
    scheduled prework items INSIDE phase D (batch-1 rsqrts batched
    back-to-back to bound ACT-table reloads).
  - v transposes are PE transposes into spare columns of the v-group's
    own PSUM slot + DVE copies (dma_start_transpose measured 1.2us per
    128x128 block of hwdge-queue time - too slow).
  - normalize muls ride the Pool (gpsimd) engine, which otherwise idles;
    po exits stay on ACT (moving any to DVE measured worse).

HW facts this relies on (probed on silicon in this session):
  - custom DVE f32->u32 output conversion is exact value truncation;
    magic-add rounding is IEEE RNE; full-tile Src1 works (only [P,1]
    broadcast Src1 is broken).
  - a matmul moving operand can be a stride-2 bf16 view of a u32 tile.
  - matmul dtype mixing 16/32-bit is rejected by walrus (hence the
    half-word trick instead of f32r).
  - shift/divide AluOps return 0 on the TRN2 DVE (hence the magic-add
    construction instead of exponent shifts).
  - gpsimd/Pool has no PSUM port; PSUM exits must use ACT/DVE.
"""

import os
os.environ.setdefault("JAX_PLATFORMS", "axon")
import sys
if "/opt/trn_rl_repo" not in sys.path:
    sys.path.insert(0, "/opt/trn_rl_repo")

import numpy as np
import ml_dtypes

import concourse.bass as bass  # noqa: F401
from concourse import bacc, mybir
import concourse.tile as tile
from concourse.bass_utils import run_bass_kernel_spmd

B, N, DIM = 2, 2048, 1024
H, DH = 16, 64
NCORES = 8
HPC = H // NCORES          # heads per core = 2
CD = HPC * DH              # per-core projection width = 128
T = B * N                  # 4096 tokens
DCH = DIM // 128           # 8 dim chunks
F32 = mybir.dt.float32
F16 = mybir.dt.float16
BF16 = mybir.dt.bfloat16
U32 = mybir.dt.uint32
AF = mybir.ActivationFunctionType
IB = 512                   # attention i-block (queries per phase-D block)
NBLK = T // IB             # 8 blocks
NJ = N // 128              # 16 key chunks per batch
NG = NBLK * NJ             # 128 granules, 64 pairs

SOFTCLAMP = 50.0
RMS_EPS = 1e-6

# ---------------------------------------------------------------------------
# Custom-op numerics (see fit_ops.py for derivation + numpy validation).
# ---------------------------------------------------------------------------
AMP = 2.0**23 * np.log2(np.e) * 6.25
TA, TB_, TC, TD = 0.99819183, -0.31795733, 0.09607557, -0.01496778
_a1 = AMP * TA / 50.0
_a3 = AMP * TB_ / 50.0**3
_a5 = AMP * TC / 50.0**5
_a7 = AMP * TD / 50.0**7
LAM = float((-_a7) ** (1.0 / 7.0))   # sigma = LAM*s ; sigma^7 coeff == -1
K5 = float(np.float32(_a5 / LAM**5))
K3 = float(np.float32(_a3 / LAM**3))
K1 = float(np.float32(_a1 / LAM))

Q0, Q1, Q2 = 1.41569374, 0.9921173, 0.3220771   # 2^(f+0.5) quad minimax
M_BIG = float(np.float32(1.5 * 2.0**46))
# +2^15 nudges the bf16 high-halfword truncation to round-to-nearest
B0 = float(np.float32(2.0**23 * (126.0 + Q0) + 32768.0))
B1 = float(np.float32(Q1))
B2 = float(np.float32(Q2 / 2.0**23))
EXP_B_BIAS = float(0.5 * np.log(2.0))   # B-path: w = exp(6.25*t + 0.5*ln2)

_OPS = None


def _register_ops():
    global _OPS
    if _OPS is not None:
        return _OPS
    import concourse.dve_ops as dve_ops
    from concourse.dve_spec import (Spec, Src0, Src1, C0, C1, C2, lower,
                                    _has_src1)
    from concourse.dve_uop import DveOpSpec

    def mk(name, body, ref):
        if name in dve_ops._SUB_OPCODE_FOR_NAME:
            return next(o for o in dve_ops.OPS if o.name == name)
        spec = Spec(body=body, reference=ref)
        row = dve_ops._CUSTOM_DVE_ROW_BASE + len(dve_ops.OPS)
        assert row < 0x20
        dve_ops._SUB_OPCODE_FOR_NAME[name] = row
        shas = {}
        for ver in ("v3", "v4"):
            uops = lower(spec, ver=ver)
            shas[ver] = DveOpSpec(name=name, opcode=row, uops=uops,
                                  rd1_en=_has_src1(spec)).sha(ver)
        op = dve_ops.DveOp(name, spec, subdim=False, uops_sha=shas)
        dve_ops.OPS.append(op)
        dve_ops.CUSTOM_DVE_SPECS[name] = op.spec
        return op

    u = Src0 * Src0
    body1 = ((((C0 - u) * u + C1) * u) + C2) * Src0

    def ref1(in0, in1, s0, s1, imm2):
        x = in0.astype(np.float32)
        uu = x * x
        return ((((s0 - uu) * uu + s1) * uu) + imm2) * x

    n23 = (Src0 + C0) - C0
    Fv = Src0 - n23
    body2 = (((Fv * C1) + C2) * Fv + Src1) + n23

    def ref2(in0, in1, s0, s1, imm2):
        Y = in0.astype(np.float32)
        nn = ((Y + s0).astype(np.float32) - s0).astype(np.float32)
        Ff = (Y - nn).astype(np.float32)
        t = ((Ff * s1).astype(np.float32) + imm2).astype(np.float32)
        t = ((t * Ff).astype(np.float32) + in1).astype(np.float32)
        return (t + nn).astype(np.float32)

    _OPS = (mk("TANH2Y_ANT", body1, ref1), mk("EXP2V_ANT", body2, ref2))
    return _OPS


def build_nc(debug_outs=False):
    op1, op2 = _register_ops()
    nc = bacc.Bacc("TRN2", target_bir_lowering=False, debug=False,
                   num_devices=NCORES)
    tok = nc.declare_dram_parameter("tok", [DIM, T], BF16, isOutput=False)
    wq = nc.declare_dram_parameter("wq", [128, DCH * CD], BF16, isOutput=False)
    wk = nc.declare_dram_parameter("wk", [128, DCH * CD], BF16, isOutput=False)
    wv = nc.declare_dram_parameter("wv", [128, DCH * CD], BF16, isOutput=False)
    wo = nc.declare_dram_parameter("wo", [CD, DIM], BF16, isOutput=False)
    # per-partition scale for the q-norm rsqrt: 1/(g2*LAM)^2
    gq = nc.declare_dram_parameter("gq", [CD, 1], F32, isOutput=False)
    out = nc.declare_dram_parameter("out", [T, DIM], BF16, isOutput=True)
    dbg = None
    if debug_outs:
        dbg = {
            "d_qT": nc.declare_dram_parameter("d_qT", [128, T], BF16, True),
            "d_kT": nc.declare_dram_parameter("d_kT", [128, T], BF16, True),
            "d_v": nc.declare_dram_parameter("d_v", [128, T // 128, 256],
                                             BF16, True),
            "d_exA": nc.declare_dram_parameter("d_exA", [128, 2048], U32, True),
            "d_exB": nc.declare_dram_parameter("d_exB", [128, 2048], BF16, True),
        }

    with tile.TileContext(nc) as tc:
        _emit(nc, tc, op1, op2, tok, wq, wk, wv, wo, gq, out, dbg)
    nc.compile()
    return nc


def _emit(nc, tc, op1, op2, tok, wq, wk, wv, wo, gq, out, dbg=None):
    with tc.tile_pool(name="const", bufs=1) as const, \
         tc.tile_pool(name="core", bufs=1) as core:

        # ---- constants / weights ----
        from concourse.masks import make_identity
        ones_bf = const.tile([128, 128], BF16, tag="ones")
        nc.vector.memset(ones_bf[:], 1.0)
        ident = const.tile([128, 128], BF16, tag="ident")
        make_identity(nc, ident[:])
        bias0 = const.tile([128, 1], F32, tag="bias0")
        nc.vector.memset(bias0[:], 0.0)
        bias_e = const.tile([128, 1], F32, tag="bias_e")
        nc.vector.memset(bias_e[:], EXP_B_BIAS)
        b0t = const.tile([128, 2 * 2 * IB], F32, tag="b0t")
        nc.vector.memset(b0t[:], B0)
        gq_sb = const.tile([128, 1], F32, tag="gq")
        nc.scalar.dma_start(out=gq_sb[:], in_=gq[:])
        wq_sb = const.tile([128, DCH, CD], BF16, tag="wq")
        wk_sb = const.tile([128, DCH, CD], BF16, tag="wk")
        wv_sb = const.tile([128, DCH, CD], BF16, tag="wv")
        # wk/wq ride the sync queue AHEAD of the token stream so the first
        # projection matmuls aren't starved behind 8 MB of tokens
        for w_dram, w_sb in ((wk, wk_sb), (wq, wq_sb)):
            nc.sync.dma_start(out=w_sb[:],
                              in_=w_dram.rearrange("p (c m) -> p c m",
                                                   c=DCH))
        nc.scalar.dma_start(out=wv_sb[:],
                            in_=wv.rearrange("p (c m) -> p c m", c=DCH))
        wo_sb = const.tile([128, DIM], BF16, tag="wo")
        nc.scalar.dma_start(out=wo_sb[:], in_=wo[:])

        # persistent tensors
        qT = core.tile([128, T], BF16, tag="qT")
        kT = core.tile([128, T], BF16, tag="kT")
        vT = core.tile([128, T], BF16, tag="vT")
        # augmented v, per 128-token chunk: [onesA(64)|vA(64)|vB(64)|onesB(64)]
        v_sb = core.tile([128, T // 128, 256], BF16, tag="v")
        nc.vector.memset(v_sb[:, :, 0:64], 1.0)
        nc.vector.memset(v_sb[:, :, 192:256], 1.0)

        tok_ch = [core.tile([128, T], BF16, tag=f"tok{ch}", name=f"tok{ch}")
                  for ch in range(DCH)]
        # batch-major sub-chunk stream: each (b, ch) arrival unleashes the
        # 8 resident projection groups' ch-matmuls for that batch, so the
        # whole q+k projection pass is DMA-paced in 8 PSUM banks.
        for b in range(B):
            nsl = slice(b * N, (b + 1) * N)
            for ch in range(DCH):
                # alternate hwdge queues: each runs at ~83% descriptor
                # efficiency, so two in parallel approach the full HBM rate
                q = nc.sync if ch % 2 == 0 else nc.scalar
                q.dma_start(out=tok_ch[ch][:, nsl],
                            in_=tok[ch * 128:(ch + 1) * 128, nsl])

        # ------------------------------------------------------------------
        # projection-group + head-norm emitters (psum pool passed per phase)
        # ------------------------------------------------------------------
        def proj_group(ps_pool, w_sb, dstT, th, name, copy_dve=False):
            tsl = slice(th * 512, (th + 1) * 512)
            pq = ps_pool.tile([128, 2 * IB], F32, tag="sim", name=name)
            for ch in range(DCH):
                nc.tensor.matmul(pq[:, 0:512], w_sb[:, ch, :],
                                 tok_ch[ch][:, tsl], start=(ch == 0),
                                 stop=True, skip_group_check=True)
            if copy_dve:
                nc.vector.tensor_copy(dstT[:, tsl], pq[:, 0:512])
            else:
                nc.scalar.activation(dstT[:, tsl], pq[:, 0:512], AF.Copy)
            return pq

        def norm_chains_batched(ps_pool, pc_pool, specs, psum_shape=2 * IB,
                                tag="sim"):
            # specs: list of (dstT, tp, is_q, name). All squ muls first,
            # then all n2 matmuls, then rsqrts, then the in-place muls —
            # each engine's FIFO pipelines instead of ping-ponging.
            # n2[p, t] = sum of squares over the head owning partition p.
            sq_tiles, rq_tiles, n2s = [], [], []
            for dstT, tp, is_q, name in specs:
                tfull = slice(tp * 1024, (tp + 1) * 1024)
                squ = pc_pool.tile([128, 1024], BF16, tag="squ")
                nc.vector.tensor_mul(squ[:], dstT[:, tfull], dstT[:, tfull])
                sq_tiles.append(squ)
            for squ, (dstT, tp, is_q, name) in zip(sq_tiles, specs):
                pair = []
                for ti in range(2):
                    csl = slice(ti * 512, (ti + 1) * 512)
                    n2 = ps_pool.tile([128, psum_shape], F32, tag=tag,
                                      name=f"{name}_{ti}")
                    nc.tensor.matmul(n2[0:64, 0:512], ones_bf[0:64, 0:64],
                                     squ[0:64, csl], start=True, stop=True)
                    nc.tensor.matmul(n2[64:128, 0:512], ones_bf[64:128, 0:64],
                                     squ[64:128, csl], start=True, stop=True)
                    pair.append(n2)
                n2s.append(pair)
            for pair, (dstT, tp, is_q, name) in zip(n2s, specs):
                rq = pc_pool.tile([128, 1024], F16, tag="rq")
                sc = gq_sb[:] if is_q else 1.0
                for ti in range(2):
                    csl = slice(ti * 512, (ti + 1) * 512)
                    nc.scalar.activation(rq[:, csl], pair[ti][:, 0:512],
                                         AF.Abs_reciprocal_sqrt,
                                         bias=bias0[:], scale=sc)
                rq_tiles.append(rq)
            for rq, (dstT, tp, is_q, name) in zip(rq_tiles, specs):
                tfull = slice(tp * 1024, (tp + 1) * 1024)
                nc.vector.tensor_mul(dstT[:, tfull], dstT[:, tfull], rq[:])

        def vproj_mm(ps_pool, th, name):
            return proj_group(ps_pool, wv_sb, vT, th, name)

        def vtr(pq, i, tv):
            # transposes ride unused columns of the group's own pq slot
            ptr = pq[:, 512 + i * 64: 512 + (i + 1) * 64].bitcast(BF16)
            nc.tensor.transpose(ptr,
                                vT[:, tv * 128:(tv + 1) * 128], ident[:])
            nc.vector.tensor_copy(v_sb[:, tv, 64:192].bitcast(U32),
                                  ptr.bitcast(U32))

        # ---- lead-in: ALL q/k projections + head norms, chunk-major with
        # 8 resident accumulation banks per batch pass, so PE work is paced
        # by the token DMA stream. Keeps the Abs_reciprocal_sqrt ACT-table
        # state out of phase D (tanh/exp/copy share one table set). ----
        # ---- lead-in: BATCH-0 q/k projections + head norms only. Batch-1
        # projections and norms drain inside phase D (its first 4 blocks
        # touch only batch 0), with their rsqrts batched back-to-back so
        # the tanh/exp ACT-table is reloaded exactly twice. ----
        with tc.tile_pool(name="psc", bufs=8, space="PSUM") as psc, \
             tc.tile_pool(name="pc0", bufs=2) as pc0:
            # PE HAM warm-up: dummy matmuls before the first token
            # sub-chunk lands, so the projection stream runs at 2.4 GHz
            warm = psc.tile([128, 512], F32, tag="pp", name="warm")
            for _ in range(30):
                nc.tensor.matmul(warm[:, 0:128], ones_bf[:], ident[:],
                                 start=True, stop=True,
                                 skip_group_check=True)
            pqk = {}
            for th in range(4):
                pqk["k", th] = psc.tile([128, 512], F32, tag="pp",
                                        name=f"pk{th}")
                pqk["q", th] = psc.tile([128, 512], F32, tag="pp",
                                        name=f"pq{th}")
            for ch in range(DCH):
                for th in range(4):
                    tsl = slice(th * 512, (th + 1) * 512)
                    nc.tensor.matmul(pqk["k", th][:], wk_sb[:, ch, :],
                                     tok_ch[ch][:, tsl], start=(ch == 0),
                                     stop=True, skip_group_check=True)
                    nc.tensor.matmul(pqk["q", th][:], wq_sb[:, ch, :],
                                     tok_ch[ch][:, tsl], start=(ch == 0),
                                     stop=True, skip_group_check=True)
            # k copies on ACT, q copies on the otherwise-idle DVE so the
            # two copy streams drain in parallel ahead of the chains
            for th in range(4):
                tsl = slice(th * 512, (th + 1) * 512)
                nc.scalar.activation(kT[:, tsl], pqk["k", th][:], AF.Copy)
                nc.vector.tensor_copy(qT[:, tsl], pqk["q", th][:])
            norm_chains_batched(
                psc, pc0,
                [(kT, 0, False, "kn0"), (qT, 0, True, "qn0"),
                 (kT, 1, False, "kn1"), (qT, 1, True, "qn1")],
                psum_shape=512, tag="pp")
            # prefetch the tanh+exp table set before phase D (Tanh then Exp
            # back-to-back forces a set containing both: exp_and_others)
            scr = pc0.tile([128, 16], F32, tag="scr")
            nc.scalar.activation(scr[:], ones_bf[:, 0:16], AF.Tanh,
                                 bias=bias0[:], scale=1.0)
            nc.scalar.activation(scr[:], ones_bf[:, 0:16], AF.Exp,
                                 bias=bias0[:], scale=1.0)

        # ---- phase D: attention, software-pipelined ----
        with tc.tile_pool(name="psO", bufs=1, space="PSUM") as psO, \
             tc.tile_pool(name="psS", bufs=3, space="PSUM") as psS, \
             tc.tile_pool(name="pc", bufs=2) as pc, \
             tc.tile_pool(name="pY", bufs=2) as pY, \
             tc.tile_pool(name="pxa", bufs=2) as pxa, \
             tc.tile_pool(name="pxb", bufs=2) as pxb, \
             tc.tile_pool(name="pe", bufs=1) as pe:

            # prework schedule: granule index -> emission item. v-groups for
            # batch 0 first (PV deadline g=5..20), then batch-1 q/k
            # projections + norms (sim deadline g=58), then batch-1 v
            # (PV deadline g=69+). Transpose/copy pairs trail their group's
            # matmuls so the DVE copy never head-blocks on a fresh
            # transpose.
            prework = {}

            def sched_vgroup(th, g0):
                # mm-group at g0, all 4 transpose+copy pairs at g0+1 (the pq
                # slot is recycled by the sim allocated 2 granules later, so
                # every read must be emitted by then)
                state = {}

                def mm():
                    state["pq"] = vproj_mm(psS, th, f"v{th}")

                def trs():
                    for i in range(4):
                        vtr(state["pq"], i, th * 4 + i)
                prework[g0] = mm
                prework[g0 + 1] = trs
                return g0 + 2

            g0 = 0
            for th in range(4):
                g0 = sched_vgroup(th, g0)
            # batch-1 k/q projection groups, 1 item per 2 granules
            for i, (w_sb, dstT, th, nm) in enumerate(
                    [(wk_sb, kT, th, f"k{th}") for th in range(4, 8)] +
                    [(wq_sb, qT, th, f"q{th}") for th in range(4, 8)]):
                prework[g0 + 2 * i] = (
                    lambda w_sb=w_sb, dstT=dstT, th=th, nm=nm:
                    proj_group(psS, w_sb, dstT, th, nm))
            g0 += 16
            # batch-1 norm chains: squ+n2 per chain staged into quarters of
            # one f16 tile, then just TWO rsqrt instructions (k-chains and
            # q-chains), then the muls
            b1chains = [(kT, 2, False), (kT, 3, False),
                        (qT, 2, True), (qT, 3, True)]
            n2all = pe.tile([128, 4096], F16, tag="n2all")
            rqall = pe.tile([128, 4096], F16, tag="rqall")
            cstate = {}

            def sched_chain_sq(ci, dstT, tp, is_q):
                def go():
                    tfull = slice(tp * 1024, (tp + 1) * 1024)
                    squ = pc.tile([128, 1024], BF16, tag="squ")
                    nc.vector.tensor_mul(squ[:], dstT[:, tfull],
                                         dstT[:, tfull])
                    n2 = psS.tile([128, 2 * IB], F32, tag="sim",
                                  name=f"n2b1_{ci}")
                    for ti in range(2):
                        csl = slice(ti * 512, (ti + 1) * 512)
                        nc.tensor.matmul(n2[0:64, csl], ones_bf[0:64, 0:64],
                                         squ[0:64, csl], start=True,
                                         stop=True)
                        nc.tensor.matmul(n2[64:128, csl],
                                         ones_bf[64:128, 0:64],
                                         squ[64:128, csl], start=True,
                                         stop=True)
                    # stage to SBUF f16 so the psS slot frees immediately;
                    # fold the q-side gamma scale here so ONE rsqrt serves
                    # all four chains (no ACT-table ping-pong)
                    dsl = slice(ci * 1024, (ci + 1) * 1024)
                    if is_q:
                        nc.vector.tensor_scalar_mul(n2all[:, dsl], n2[:],
                                                    gq_sb[:])
                    else:
                        nc.vector.tensor_copy(n2all[:, dsl], n2[:])
                return go

            def sched_chain_rsqrts():
                nc.scalar.activation(rqall[:], n2all[:],
                                     AF.Abs_reciprocal_sqrt,
                                     bias=bias0[:], scale=1.0)

            def sched_chain_mul(ci, dstT, tp):
                def go():
                    tfull = slice(tp * 1024, (tp + 1) * 1024)
                    nc.vector.tensor_mul(
                        dstT[:, tfull], dstT[:, tfull],
                        rqall[:, ci * 1024:(ci + 1) * 1024])
                return go

            for ci, (dstT, tp, is_q) in enumerate(b1chains):
                prework[g0 + ci] = sched_chain_sq(ci, dstT, tp, is_q)
            g0 += 5
            prework[g0] = sched_chain_rsqrts
            g0 += 2
            for ci, (dstT, tp, is_q) in enumerate(b1chains):
                prework[g0 + ci] = sched_chain_mul(ci, dstT, tp)
            g0 += 4
            for th in range(4, 8):
                g0 = sched_vgroup(th, g0)

            sims = [None] * NG
            Ypair = [None] * (NG // 2)
            expair = [None] * (NG // 2)   # (kind, tile)
            outp = {}
            pending = []

            def is_A_pair(pp):
                return pp % 2 == 0

            def is_C_pair(pp):
                return False

            def ioff_of(blk):
                b, iq = blk // 4, blk % 4
                return b * N + iq * IB

            def emit_sim(g):
                blk, j = divmod(g, NJ)
                boff = (blk // 4) * N
                ioff = ioff_of(blk)
                jsl = slice(boff + j * 128, boff + (j + 1) * 128)
                isl = slice(ioff, ioff + IB)
                sim = psS.tile([128, 2 * IB], F32, tag="sim", name="sim")
                nc.tensor.matmul(sim[:, 0:IB], kT[0:64, jsl],
                                 qT[0:64, isl], start=True, stop=True)
                nc.tensor.matmul(sim[:, IB:2 * IB], kT[64:128, jsl],
                                 qT[64:128, isl], start=True, stop=True)
                sims[g] = sim

            def emit_stage1(g):
                pp = g // 2
                sim = sims[g]
                if g % 2 == 0:
                    Ypair[pp] = pY.tile([128, 2 * 2 * IB], F32, tag="Y",
                                        name="Y")
                dst = Ypair[pp][:, (g % 2) * 2 * IB:(g % 2 + 1) * 2 * IB]
                if is_A_pair(pp) or is_C_pair(pp):
                    nc.vector._custom_dve(op1, out=dst, in0=sim[:],
                                          s0=K5, s1=K3, imm2=K1)
                else:
                    nc.scalar.activation(dst, sim[:], AF.Tanh,
                                         bias=bias0[:],
                                         scale=1.0 / (SOFTCLAMP * LAM))
                sims[g] = None

            def emit_stage2(pp):
                Yp = Ypair[pp]
                if is_A_pair(pp):
                    ex = pxa.tile([128, 2 * 2 * IB], F32, tag="exA",
                                  name="exA")
                    nc.vector._custom_dve(op2, out=ex[:].bitcast(U32),
                                          in0=Yp[:], in1=b0t[:],
                                          s0=M_BIG, s1=B2, imm2=B1)
                    expair[pp] = ("A", ex)
                elif is_C_pair(pp):
                    # Yp holds op1's 2^23-scaled log2 value; exp it on ACT
                    ex = pxb.tile([128, 2 * 2 * IB], BF16, tag="exB",
                                  name="exB")
                    nc.scalar.activation(ex[:], Yp[:], AF.Exp,
                                         bias=bias_e[:],
                                         scale=float(np.log(2.0) / 2.0**23))
                    expair[pp] = ("B", ex)
                else:
                    ex = pxb.tile([128, 2 * 2 * IB], BF16, tag="exB",
                                  name="exB")
                    nc.scalar.activation(ex[:], Yp[:], AF.Exp,
                                         bias=bias_e[:], scale=6.25)
                    expair[pp] = ("B", ex)
                Ypair[pp] = None
                if dbg and pp <= 1:
                    which = "d_exA" if is_A_pair(pp) else "d_exB"
                    src = expair[pp][1][:]
                    if is_A_pair(pp):
                        src = src.bitcast(U32)
                    nc.sync.dma_start(out=dbg[which][:], in_=src)

            def ex_view(g, half):
                # [128, IB] moving-operand view of granule g's weights for
                # head `half` (0=A cols 0:IB, 1=B cols IB:2IB)
                kind, ex = expair[g // 2]
                off = (g % 2) * 2 * IB + half * IB
                if kind == "B":
                    return ex[:, off:off + IB]
                ap = ex[:, off:off + IB].bitcast(BF16)
                return ap.rearrange("p (n two) -> p n two", two=2)[:, :, 1]

            def emit_pv(g):
                blk, j = divmod(g, NJ)
                if j == 0:
                    outp[blk] = (
                        psO.tile([128, IB], F32, tag="outA", name="outA"),
                        psO.tile([128, IB], F32, tag="outB", name="outB"))
                outpA, outpB = outp[blk]
                jv = (blk // 4) * NJ + j
                st = (j == 0)
                sp = (j == NJ - 1)
                nc.tensor.matmul(outpA[:, 0:IB], v_sb[:, jv, 0:128],
                                 ex_view(g, 0), start=st, stop=sp)
                nc.tensor.matmul(outpB[:, 0:IB], v_sb[:, jv, 128:256],
                                 ex_view(g, 1), start=st, stop=sp)
                if g % 2 == 1:
                    expair[g // 2] = None

            def finish_exits(blk, last=False):
                outpA, outpB = outp.pop(blk)
                # denA = outpA[0:64], vA-out = outpA[64:128];
                # vB-out = outpB[0:64], denB = outpB[64:128].
                ra = pe.tile([128, IB], F32, tag="ra")
                rb = pe.tile([128, IB], F32, tag="rb")
                tmpA = pe.tile([128, IB], BF16, tag="tmpA")
                tmpB = pe.tile([128, IB], BF16, tag="tmpB")
                nc.vector.reciprocal_approx_fast(ra[:], outpA[:])
                nc.vector.tensor_copy(tmpA[64:128, :], outpA[64:128, :])
                nc.vector.reciprocal_approx_fast(rb[:], outpB[:])
                nc.vector.tensor_copy(tmpB[0:64, :], outpB[0:64, :])
                rs = pe.tile([128, IB], F32, tag="rs")
                nc.gpsimd.dma_start(out=rs[64:128, :], in_=ra[0:64, :])
                nc.gpsimd.dma_start(out=rs[0:64, :], in_=rb[64:128, :])

                attT = pe.tile([128, IB], BF16, tag="attT")

                def normalize():
                    # rows [0:64] = head B dims, [64:128] = head A dims
                    # (wo is host-reordered to match). Pool in steady
                    # state; the final block drains on the idle DVE
                    eng = nc.vector if last else nc.gpsimd
                    eng.tensor_mul(attT[0:64, :], tmpB[0:64, :],
                                   rs[0:64, :])
                    eng.tensor_mul(attT[64:128, :], tmpA[64:128, :],
                                   rs[64:128, :])

                pending.append(normalize)

                o_big = pe.tile([128, IB // 128, DIM], BF16, tag="obig")
                ioff = ioff_of(blk)

                def mk(tci):
                    def go():
                        po = psS.tile([128, DIM], F32, tag="sim", name="po")
                        tsl = slice(tci * 128, (tci + 1) * 128)
                        for ec in range(2):
                            esl = slice(ec * 512, (ec + 1) * 512)
                            nc.tensor.matmul(po[:, esl], attT[:, tsl],
                                             wo_sb[:, esl], start=True,
                                             stop=True)
                        if last and tci % 2 == 1:
                            nc.vector.tensor_copy(o_big[:, tci, :], po[:])
                        else:
                            nc.scalar.activation(o_big[:, tci, :], po[:],
                                                 AF.Copy)
                        r0 = ioff + tci * 128
                        q = nc.scalar if (last and tci % 2 == 0) else nc.sync
                        q.dma_start(out=out[r0:r0 + 128, :],
                                    in_=o_big[:, tci, :])
                    return go

                for tci in range(IB // 128):
                    pending.append(mk(tci))

            # re-warm the PE across the lead-in chain latency gap
            warm2 = psS.tile([128, 2 * IB], F32, tag="sim", name="warm2")
            for _ in range(8):
                nc.tensor.matmul(warm2[:, 0:128], ones_bf[:], ident[:],
                                 start=True, stop=True,
                                 skip_group_check=True)
            emit_sim(0)
            emit_sim(1)
            emit_sim(2)
            for g in range(NG):
                emit_stage1(g)
                if g % 2 == 1:
                    emit_stage2(g // 2)
                if g in prework:
                    prework.pop(g)()
                elif pending and (len(pending) > 4 or g % 3 == 2):
                    pending.pop(0)()
                if g + 3 < NG:
                    emit_sim(g + 3)
                if g % 2 == 1 and g >= 3:
                    emit_pv(g - 3)
                    emit_pv(g - 2)
                if g % NJ == 6 and g > NJ:
                    finish_exits(g // NJ - 1)
            for gg in range(NG - 2, NG):
                emit_pv(gg)
            finish_exits(NBLK - 1, last=True)
            for fn in pending:
                fn()
            if dbg:
                nc.sync.dma_start(out=dbg["d_qT"][:], in_=qT[:])
                nc.sync.dma_start(out=dbg["d_kT"][:], in_=kT[:])
                nc.sync.dma_start(out=dbg["d_v"][:], in_=v_sb[:])


_NC = None


def _get_nc():
    global _NC
    if _NC is None:
        _NC = build_nc()
    return _NC


def _ensure_axon_hooks():
    try:
        import antenv.axon_hooks  # noqa: F401
        return
    except ImportError:
        pass
    import types
    hook = None
    try:
        if "/root/.axon_site" not in sys.path:
            sys.path.insert(0, "/root/.axon_site")
        from trn_agent_boot.trn_boot import _ntff_profile_via_ctypes
        hook = _ntff_profile_via_ctypes("/opt/axon/libaxon_pjrt.so")
    except Exception:
        hook = None
    m = types.ModuleType("antenv.axon_hooks")
    m.get_axon_ntff_profile_hook = lambda: hook
    sys.modules["antenv.axon_hooks"] = m


def kernel(tokens, norm_w, w_q, w_kv, w_out, q_gamma, k_gamma):
    tokens = np.asarray(tokens, np.float32)
    norm_w = np.asarray(norm_w, np.float32)
    w_q = np.asarray(w_q, np.float32)
    w_kv = np.asarray(w_kv, np.float32)
    w_out = np.asarray(w_out, np.float32)
    q_gamma = np.asarray(q_gamma, np.float32)
    k_gamma = np.asarray(k_gamma, np.float32)

    bf = ml_dtypes.bfloat16
    # host-side rmsnorm scale folded into the tokens (f32, exact)
    tok2 = tokens.reshape(T, DIM)
    s = 1.0 / np.sqrt((tok2 * tok2).mean(axis=1, keepdims=True) + RMS_EPS)
    tok_n = tok2 * s
    tok_bf = np.ascontiguousarray(tok_n.astype(bf).T)

    wq_f = norm_w[:, None] * w_q
    wkv_f = norm_w[:, None] * w_kv
    wk_f = wkv_f[:, :H * DH]
    wv_f = wkv_f[:, H * DH:]
    # combined q*k gamma scale (incl. both sqrt(DH) factors) and the custom
    # op's LAM pre-scale, applied on the q side
    g2_full = ((q_gamma + 1.0) * (k_gamma + 1.0) * float(DH)).reshape(H * DH)

    def _swz(w):
        return np.ascontiguousarray(
            w.astype(bf).reshape(DCH, 128, CD).transpose(1, 0, 2)
            .reshape(128, DCH * CD))

    in_maps = []
    for c in range(NCORES):
        cols = slice(c * CD, (c + 1) * CD)
        g2c = g2_full[c * CD:(c + 1) * CD] * LAM
        wo_c = w_out[cols, :]
        # attT rows are [head B dims, head A dims]
        wo_r = np.concatenate([wo_c[64:128, :], wo_c[0:64, :]], axis=0)
        in_maps.append({
            "tok": tok_bf,
            "wq": _swz(wq_f[:, cols]),
            "wk": _swz(wk_f[:, cols]),
            "wv": _swz(wv_f[:, cols]),
            "wo": np.ascontiguousarray(wo_r).astype(bf),
            "gq": np.ascontiguousarray(
                (1.0 / (g2c * g2c)).reshape(CD, 1), dtype=np.float32),
        })

    nc = _get_nc()
    trace = os.environ.get("KBENCH_TRACE") == "1"
    kwargs = {}
    if trace:
        _ensure_axon_hooks()
        import concourse.bass_utils as _bu
        _bu.upload_artifacts = lambda d: "local://" + d
        kwargs = {"trace": True,
                  "tmpdir": os.environ.get("KBENCH_TRACE_DIR") or None}
    res = run_bass_kernel_spmd(nc, in_maps, core_ids=list(range(NCORES)),
                               **kwargs)
    if res.exec_time_ns is not None:
        print(f"HW exec time: {res.exec_time_ns} ns")
    acc = np.zeros((T, DIM), np.float32)
    for i in range(NCORES):
        acc += res.results[i]["out"].astype(np.float32)
    return acc.reshape(B, N, DIM)


if __name__ == "__main__":
    rng = np.random.default_rng(0)
    inputs = {
        "tokens": rng.standard_normal((B, N, DIM), dtype=np.float32),
        "norm_w": np.ones((DIM,), np.float32),
        "w_q": rng.standard_normal((DIM, H * DH), dtype=np.float32) * 0.02,
        "w_kv": rng.standard_normal((DIM, 2 * H * DH), dtype=np.float32) * 0.02,
        "w_out": rng.standard_normal((H * DH, DIM), dtype=np.float32) * 0.02,
        "q_gamma": np.zeros((H, DH), np.float32),
        "k_gamma": np.zeros((H, DH), np.float32),
    }
    out = kernel(**inputs)
    print("out", out.shape, out.dtype, float(np.abs(out).max()))
